# revision 1
# baseline (speedup 1.0000x reference)
"""AttnBlock (GroupNorm -> single-head attention over 64x64 tokens -> proj -> residual)
for Trainium2, SPMD over 8 NeuronCores.

Sharding: core = batch(4) x query-half(2).  Each core receives x[b] with its
query half rotated to the front (token order along j is permutation-invariant
for softmax-attention and for GroupNorm stats), computes GroupNorm + k/vT over
all 4096 tokens, q over its 2048 tokens, streaming-softmax attention without
max-subtraction (logits bounded ~7), and the output projection + residual for
its 2048 tokens.

All matmuls run in bf16 (fp32 PSUM accumulation); measured end-to-end L2 rel
err vs the fp32 reference ~3e-4.

Layouts (SBUF, partition dim first):
  h, k : [128, 4cc, 4096]  channel on partitions (4 chunks of 128), tokens free
  q    : [128, 4cc, 2048]
  vT   : [128jc, 32, 512]  token chunk on partitions, channel free
  S^T  : psum [128 j, 512 i] = sum_c k[c,j] q[c,i]  (no transposes anywhere)
  O    : psum [128 c, 512 i] = sum_j vT[j,c] * exp(S^T[j,i]), then / l_i
"""

import math
import numpy as np
import ml_dtypes

import concourse.bass as bass
import concourse.mybir as mybir
import concourse.tile as tile

P = 128
C = 512
NCC = C // P          # 4 channel chunks
HW = 4096             # tokens per batch image
IHALF = 2048          # query tokens per core
NBLK = IHALF // 512   # 4 i-blocks of 512
NJC = HW // P         # 32 j chunks of 128
NJT = HW // 512       # 8 j tiles of 512
GS = 16               # channels per group
EPS = 1e-6
INV_SQRT_C = 1.0 / math.sqrt(C)

F32 = mybir.dt.float32
BF16 = mybir.dt.bfloat16
BF = ml_dtypes.bfloat16


def _split_excess_waits(nc):
    """walrus in this container accepts only ONE sync-wait per instruction;
    move extra waits onto same-engine NOPs placed immediately before."""
    for fn in nc.m.functions:
        for bb in fn.blocks:
            insts = list(bb.instructions)
            out = []
            changed = False
            for inst in insts:
                si = inst.sync_info
                if si is not None and len(si.on_wait) > 1:
                    waits = list(si.on_wait)
                    for k, w in enumerate(waits[:-1]):
                        nop = mybir.InstNoOp(
                            name=f"{inst.name}-ws{k}",
                            sync_info=mybir.SyncInfo(on_wait=[w], on_update=[]),
                            bass_nofuse=True,
                            engine=inst.engine,
                        )
                        out.append(nop)
                    inst.sync_info = mybir.SyncInfo(
                        on_wait=[waits[-1]], on_update=list(si.on_update)
                    )
                    changed = True
                out.append(inst)
            if changed:
                bb.instructions = out


def build_nc(split_waits=True):
    nc = bass.Bass()

    x_d = nc.declare_dram_parameter("x_bc", [C, HW], F32, isOutput=False)
    xb_d = nc.declare_dram_parameter("x_bf", [C, HW], BF16, isOutput=False)
    wqt_d = nc.declare_dram_parameter("wqt", [C, C], BF16, isOutput=False)
    wkt_d = nc.declare_dram_parameter("wkt", [C, C], BF16, isOutput=False)
    wvt_d = nc.declare_dram_parameter("wvt", [C, C], BF16, isOutput=False)
    wpt_d = nc.declare_dram_parameter("wpt", [C, C], BF16, isOutput=False)
    bq_d = nc.declare_dram_parameter("bq_pc", [P, NCC], F32, isOutput=False)
    bk_d = nc.declare_dram_parameter("bk_pc", [P, NCC], F32, isOutput=False)
    bp_d = nc.declare_dram_parameter("bp_pc", [P, NCC], F32, isOutput=False)
    gamma_d = nc.declare_dram_parameter("gamma_pc", [P, NCC], F32, isOutput=False)
    beta_d = nc.declare_dram_parameter("beta_pc", [P, NCC], F32, isOutput=False)
    bv_d = nc.declare_dram_parameter("bv_row", [1, C], F32, isOutput=False)
    ind16_d = nc.declare_dram_parameter("ind16", [P, P // GS], F32, isOutput=False)
    ind16b_d = nc.declare_dram_parameter("ind16b", [P, P // GS], BF16, isOutput=False)
    bcast16_d = nc.declare_dram_parameter("bcast16", [P // GS, P], F32, isOutput=False)
    ones_d = nc.declare_dram_parameter("ones_col", [P, 1], BF16, isOutput=False)
    y_d = nc.declare_dram_parameter("yout", [C, IHALF], F32, isOutput=True)

    with tile.TileContext(nc) as tc:
        # ---- persistent pools (live through the whole kernel) ----
        with (
            tc.tile_pool(name="w", bufs=1) as wpool,
            tc.tile_pool(name="const", bufs=1) as cpool,
            tc.tile_pool(name="kbuf", bufs=1) as kpool,
            tc.tile_pool(name="vbuf", bufs=1) as vpool,
            tc.tile_pool(name="qbuf", bufs=1) as qpool,
        ):
            wqt = wpool.tile([P, NCC, C], BF16, tag="wqt")
            wkt = wpool.tile([P, NCC, C], BF16, tag="wkt")
            wvt = wpool.tile([P, NCC, C], BF16, tag="wvt")
            wpt = wpool.tile([P, NCC, C], BF16, tag="wpt")
            wdmas = [(t, d) for t, d in ((wqt, wqt_d), (wkt, wkt_d), (wvt, wvt_d), (wpt, wpt_d))]

            bq_sb = cpool.tile([P, NCC], F32, tag="bq")
            bk_sb = cpool.tile([P, NCC], F32, tag="bk")
            bp_sb = cpool.tile([P, NCC], F32, tag="bp")
            gamma_sb = cpool.tile([P, NCC], F32, tag="gamma")
            beta_sb = cpool.tile([P, NCC], F32, tag="beta")
            ind16_sb = cpool.tile([P, P // GS], F32, tag="ind16")
            ind16b_sb = cpool.tile([P, P // GS], BF16, tag="ind16b")
            bcast16_sb = cpool.tile([P // GS, P], F32, tag="bcast16")
            ones_f = cpool.tile([P, 1], F32, tag="onesf")
            bv_sb = cpool.tile([P, C], F32, tag="bvb")
            eps_sb = cpool.tile([P // GS, 1], F32, tag="eps")
            cdmas = [
                (gamma_sb, gamma_d), (beta_sb, beta_d),
                (bq_sb, bq_d), (bk_sb, bk_d), (bp_sb, bp_d),
            ]
            nc.gpsimd.dma_start(out=ind16_sb[:], in_=ind16_d[:])
            nc.gpsimd.dma_start(out=ind16b_sb[:], in_=ind16b_d[:])
            nc.gpsimd.dma_start(out=bcast16_sb[:], in_=bcast16_d[:])
            nc.vector.memset(eps_sb[:], EPS)
            nc.vector.memset(ones_f[:], 1.0)

            k_sb = kpool.tile([P, NCC, HW], BF16, tag="k")
            vt_sb = vpool.tile([P, NJC, C], BF16, tag="vt")
            q_sb = qpool.tile([P, NCC, IHALF], BF16, tag="q")

            # ====== phase 0: stream x once (bf16) -> GN stats -> h in place ======
            with (
                tc.tile_pool(name="hbuf", bufs=1) as hpool,
                tc.tile_pool(name="gn", bufs=2) as gpool,
            ):
                # holds bf16(x), overwritten in place by h = x*scale + shift
                h_sb = hpool.tile([P, NCC, HW], BF16, tag="h")

                half = HW // 2
                for ci, eng in ((0, nc.sync), (3, nc.gpsimd), (1, nc.sync), (2, nc.sync)):
                    eng.dma_start(out=h_sb[:, ci, :half], in_=xb_d[ci * P:(ci + 1) * P, :half])
                    eng.dma_start(out=h_sb[:, ci, half:], in_=xb_d[ci * P:(ci + 1) * P, half:])
                for t, d in cdmas:
                    nc.gpsimd.dma_start(out=t[:], in_=d[:])
                nc.gpsimd.dma_start(out=bv_sb[:], in_=bv_d[:].to_broadcast((P, C)))
                for t, d in wdmas:
                    nc.sync.dma_start(out=t[:], in_=d[:].rearrange("(cc p) o -> p cc o", p=P))

                scale_sb = gpool.tile([P, NCC], F32, tag="scale")
                shift_sb = gpool.tile([P, NCC], F32, tag="shift")
                with tc.tile_pool(name="gnp", bufs=2, space="PSUM") as gpsum_pool:
                    gpsum = gpsum_pool.tile([P // GS, 2 * NCC], F32, tag="gstat")
                    for ci in range(NCC):
                        t2 = gpool.tile([P, 2], F32, tag="t2")
                        if ci in (0, 2):
                            stats = gpool.tile([P, HW // 512, 6], F32, tag="stats")
                            for sg in range(HW // 512):
                                nc.vector.bn_stats(
                                    out=stats[:, sg, :],
                                    in_=h_sb[:, ci, sg * 512:(sg + 1) * 512],
                                )
                            mv = gpool.tile([P, 2], F32, tag="mv")
                            nc.vector.bn_aggr(out=mv[:], in_=stats[:])
                            nc.vector.tensor_copy(out=t2[:, 0:1], in_=mv[:, 0:1])
                            nc.vector.tensor_tensor(
                                t2[:, 1:2], mv[:, 0:1], mv[:, 0:1], mybir.AluOpType.mult
                            )
                            nc.vector.tensor_add(t2[:, 1:2], t2[:, 1:2], mv[:, 1:2])
                        else:
                            s1 = gpool.tile([P, 1], F32, tag="s1")
                            s2 = gpool.tile([P, 1], F32, tag="s2")
                            scr = gpool.tile([P, HW], BF16, tag="scr")
                            nc.scalar.activation(
                                out=scr[:], in_=h_sb[:, ci, :],
                                func=mybir.ActivationFunctionType.Copy, accum_out=s1[:],
                            )
                            nc.scalar.activation(
                                out=scr[:], in_=h_sb[:, ci, :],
                                func=mybir.ActivationFunctionType.Square, accum_out=s2[:],
                            )
                            nc.vector.tensor_scalar_mul(t2[:, 0:1], s1[:], 1.0 / HW)
                            nc.vector.tensor_scalar_mul(t2[:, 1:2], s2[:], 1.0 / HW)
                        nc.tensor.matmul(
                            gpsum[:, ci * 2:(ci + 1) * 2], lhsT=ind16_sb[:], rhs=t2[:],
                            start=True, stop=True,
                        )

                    # per-chunk: group mean/rstd -> broadcast -> scale/shift -> h
                    for ci in range(NCC):
                        gmr = gpool.tile([P // GS, 2], F32, tag="gmr", name=f"gmr{ci}")
                        nc.vector.tensor_copy(out=gmr[:], in_=gpsum[:, ci * 2:(ci + 1) * 2])
                        mu = gmr[:, 0:1]
                        var = gmr[:, 1:2]
                        tmpv = gpool.tile([P // GS, 1], F32, tag="tmpv")
                        nc.vector.tensor_tensor(tmpv[:], mu, mu, mybir.AluOpType.mult)
                        nc.vector.tensor_tensor(var, var, tmpv[:], mybir.AluOpType.subtract)
                        nc.scalar.activation(
                            out=var, in_=var, func=mybir.ActivationFunctionType.Sqrt,
                            bias=eps_sb[:], scale=1.0,
                        )
                        nc.vector.reciprocal(out=var, in_=var)
                        bpsum = gpsum_pool.tile([P, 2], F32, tag="bc")
                        nc.tensor.matmul(
                            bpsum[:], lhsT=bcast16_sb[:], rhs=gmr[:],
                            start=True, stop=True,
                        )
                        sc = scale_sb[:, ci:ci + 1]
                        sh = shift_sb[:, ci:ci + 1]
                        nc.vector.tensor_tensor(
                            sc, bpsum[:, 1:2], gamma_sb[:, ci:ci + 1], mybir.AluOpType.mult
                        )
                        nc.vector.tensor_tensor(sh, bpsum[:, 0:1], sc, mybir.AluOpType.mult)
                        nc.vector.tensor_tensor(
                            sh, beta_sb[:, ci:ci + 1], sh, mybir.AluOpType.subtract
                        )
                        # h in place: DVE except c3 on ACT
                        if ci != 3:
                            nc.vector.tensor_scalar(
                                out=h_sb[:, ci, :], in0=h_sb[:, ci, :],
                                scalar1=sc, scalar2=sh,
                                op0=mybir.AluOpType.mult, op1=mybir.AluOpType.add,
                            )
                        else:
                            nc.scalar.activation(
                                out=h_sb[:, ci, :], in_=h_sb[:, ci, :],
                                func=mybir.ActivationFunctionType.Identity,
                                bias=sh, scale=sc,
                            )

                with tc.tile_pool(name="mmp", bufs=4, space="PSUM") as mmpool:
                    # k[o, j] (all tokens)
                    for oc in range(NCC):
                        for jt in range(NJT):
                            ps = mmpool.tile([P, 512], F32, tag="mm")
                            for cc in range(NCC):
                                nc.tensor.matmul(
                                    ps[:],
                                    lhsT=wkt[:, cc, oc * P:(oc + 1) * P],
                                    rhs=h_sb[:, cc, jt * 512:(jt + 1) * 512],
                                    start=(cc == 0), stop=(cc == NCC - 1),
                                )
                            nc.scalar.activation(
                                out=k_sb[:, oc, jt * 512:(jt + 1) * 512], in_=ps[:],
                                func=mybir.ActivationFunctionType.Identity,
                                bias=bk_sb[:, oc:oc + 1], scale=1.0,
                            )
                    # vT[j, c] (all tokens)
                    for jc in range(NJC):
                        ps = mmpool.tile([P, 512], F32, tag="mm")
                        for cc in range(NCC):
                            nc.tensor.matmul(
                                ps[:],
                                lhsT=h_sb[:, cc, jc * P:(jc + 1) * P],
                                rhs=wvt[:, cc, :],
                                start=(cc == 0), stop=(cc == NCC - 1),
                            )
                        nc.vector.tensor_add(vt_sb[:, jc, :], ps[:], bv_sb[:])
                    # q[o, i] (this core's half)
                    for oc in range(NCC):
                        for it in range(IHALF // 512):
                            ps = mmpool.tile([P, 512], F32, tag="mm")
                            for cc in range(NCC):
                                nc.tensor.matmul(
                                    ps[:],
                                    lhsT=wqt[:, cc, oc * P:(oc + 1) * P],
                                    rhs=h_sb[:, cc, it * 512:(it + 1) * 512],
                                    start=(cc == 0), stop=(cc == NCC - 1),
                                )
                            nc.scalar.activation(
                                out=q_sb[:, oc, it * 512:(it + 1) * 512], in_=ps[:],
                                func=mybir.ActivationFunctionType.Identity,
                                bias=bq_sb[:, oc:oc + 1], scale=1.0,
                            )

            # ====== phase 2: attention per 512-token block (proj deferred) ======
            with (
                tc.tile_pool(name="et", bufs=4) as etpool,
                tc.tile_pool(name="ob", bufs=NBLK) as obpool,
                tc.tile_pool(name="la", bufs=2) as lapool,
                tc.tile_pool(name="lb", bufs=2) as lbpool,
                tc.tile_pool(name="lrbp", bufs=NBLK) as lrbpool,
                tc.tile_pool(name="ld", bufs=2, space="DRAM") as ldpool,
                tc.tile_pool(name="stp", bufs=3, space="PSUM") as stpool,
                tc.tile_pool(name="oap", bufs=1, space="PSUM") as oapool,
                tc.tile_pool(name="lp", bufs=1, space="PSUM") as lpool,
            ):
                o_bfs = []
                lrbs = []
                for ib in range(NBLK):
                    isl = slice(ib * 512, (ib + 1) * 512)
                    opsum = [
                        oapool.tile([P, 512], F32, tag=f"o{cc}", name=f"opsum{cc}")
                        for cc in range(NCC)
                    ]
                    lacc = lapool.tile([P, 512], F32, tag="lacc")
                    ets = [None] * NJC

                    def emit_st(jc):
                        ps = stpool.tile([P, 512], F32, tag="st")
                        for cc in range(NCC):
                            nc.tensor.matmul(
                                ps[:],
                                lhsT=k_sb[:, cc, jc * P:(jc + 1) * P],
                                rhs=q_sb[:, cc, isl],
                                start=(cc == 0), stop=(cc == NCC - 1),
                            )
                        et = etpool.tile([P, 512], BF16, tag="et")
                        nc.scalar.activation(
                            out=et[:], in_=ps[:],
                            func=mybir.ActivationFunctionType.Exp, scale=INV_SQRT_C,
                        )
                        ets[jc] = et

                    def emit_av(jc):
                        et = ets[jc]
                        for cc in range(NCC):
                            nc.tensor.matmul(
                                opsum[cc][:],
                                lhsT=vt_sb[:, jc, cc * P:(cc + 1) * P],
                                rhs=et[:],
                                start=(jc == 0), stop=(jc == NJC - 1),
                            )
                        # softmax denominator: accumulate exp sums on DVE
                        if jc == 0:
                            nc.vector.tensor_copy(out=lacc[:], in_=et[:])
                        else:
                            nc.vector.tensor_add(lacc[:], lacc[:], et[:])
                        ets[jc] = None

                    DEPTH = 3
                    for jc in range(DEPTH):
                        emit_st(jc)
                    for jc in range(DEPTH, NJC):
                        emit_st(jc)
                        emit_av(jc - DEPTH)
                    for jc in range(NJC - DEPTH, NJC):
                        emit_av(jc)

                    # unnormalized O -> bf16 (releases psum banks asap);
                    # 1/l is applied to the projection output in phase 3
                    o_bf = obpool.tile([P, NCC, 512], BF16, tag="obf", name=f"o_bf{ib}")
                    for cc in range(NCC):
                        nc.vector.tensor_copy(out=o_bf[:, cc, :], in_=opsum[cc][:])
                    o_bfs.append(o_bf)

                    # l = column sums of lacc via a single fp32 matmul
                    lpsum = lpool.tile([1, 512], F32, tag="l")
                    nc.tensor.matmul(
                        lpsum[:], lhsT=ones_f[:], rhs=lacc[:], start=True, stop=True
                    )
                    l_sb = lbpool.tile([1, 512], F32, tag="lsb")
                    nc.vector.reciprocal(out=l_sb[:], in_=lpsum[:])
                    l_dram = ldpool.tile([1, 512], F32, tag="ldram")
                    nc.sync.dma_start(out=l_dram[:], in_=l_sb[:])
                    lrb = lrbpool.tile([P, 512], F32, tag="lrb", name=f"lrb{ib}")
                    nc.sync.dma_start(out=lrb[:], in_=l_dram[:].to_broadcast((P, 512)))
                    lrbs.append(lrb)

                # ====== phase 3: out = Wp @ O + bp + x ======
                with (
                    tc.tile_pool(name="xr", bufs=4) as xrpool,
                    tc.tile_pool(name="os", bufs=4) as ospool,
                ):
                    for ib in range(NBLK):
                        isl = slice(ib * 512, (ib + 1) * 512)
                        o_bf = o_bfs[ib]
                        for oc in range(NCC):
                            xr = xrpool.tile([P, 512], F32, tag="xr")
                            nc.gpsimd.dma_start(
                                out=xr[:], in_=x_d[oc * P:(oc + 1) * P, isl]
                            )
                            # xr += bp on the otherwise-idle GpSimd engine
                            nc.gpsimd.tensor_scalar(
                                out=xr[:], in0=xr[:], scalar1=bp_sb[:, oc:oc + 1],
                                scalar2=None, op0=mybir.AluOpType.add,
                            )
                            ps = stpool.tile([P, 512], F32, tag="st")
                            for cc in range(NCC):
                                nc.tensor.matmul(
                                    ps[:],
                                    lhsT=wpt[:, cc, oc * P:(oc + 1) * P],
                                    rhs=o_bf[:, cc, :],
                                    start=(cc == 0), stop=(cc == NCC - 1),
                                )
                            ost = ospool.tile([P, 512], F32, tag="ost")
                            nc.vector.tensor_tensor(
                                ost[:], ps[:], lrbs[ib][:], mybir.AluOpType.mult
                            )
                            nc.vector.tensor_add(ost[:], ost[:], xr[:])
                            nc.scalar.dma_start(out=y_d[oc * P:(oc + 1) * P, isl], in_=ost[:])

    if split_waits:
        _split_excess_waits(nc)
    return nc


_NC = None


def _get_nc():
    global _NC
    if _NC is None:
        _NC = build_nc()
    return _NC


def _core0_feed(inputs):
    """Input map for core 0 (batch 0, first query half) — used by test harnesses."""
    maps = _build_in_maps(**inputs)
    return maps[0]


def _build_in_maps(x, gamma, beta, Wq, bq, Wk, bk, Wv, bv, Wp, bp):
    x = np.asarray(x, dtype=np.float32)
    B, c, H, W = x.shape
    assert (B, c, H, W) == (4, C, 64, 64)

    def pc(v):  # [C] -> [P, NCC]
        return np.ascontiguousarray(np.asarray(v, np.float32).reshape(NCC, P).T)

    ind16 = np.zeros((P, P // GS), np.float32)
    ind16[np.arange(P), np.arange(P) // GS] = 1.0 / GS
    bcast16 = np.zeros((P // GS, P), np.float32)
    bcast16[np.arange(P) // GS, np.arange(P)] = 1.0

    shared = {
        "wqt": np.ascontiguousarray(np.asarray(Wq, np.float32).T).astype(BF),
        "wkt": np.ascontiguousarray(np.asarray(Wk, np.float32).T).astype(BF),
        "wvt": np.ascontiguousarray(np.asarray(Wv, np.float32).T).astype(BF),
        "wpt": np.ascontiguousarray(np.asarray(Wp, np.float32).T).astype(BF),
        "bq_pc": pc(bq), "bk_pc": pc(bk), "bp_pc": pc(bp),
        "gamma_pc": pc(gamma), "beta_pc": pc(beta),
        "bv_row": np.ascontiguousarray(np.asarray(bv, np.float32).reshape(1, C)),
        "ind16": ind16, "ind16b": ind16.astype(BF), "bcast16": bcast16,
        "ones_col": np.ones((P, 1), BF),
    }

    xf = x.reshape(B, C, HW)
    in_maps = []
    for core in range(8):
        b, half = divmod(core, 2)
        xb = xf[b]
        if half == 0:
            x_bc = xb
        else:
            x_bc = np.concatenate([xb[:, IHALF:], xb[:, :IHALF]], axis=1)
        x_bc = np.ascontiguousarray(x_bc)
        in_maps.append({"x_bc": x_bc, "x_bf": x_bc.astype(BF), **shared})
    return in_maps


def kernel(x, gamma, beta, Wq, bq, Wk, bk, Wv, bv, Wp, bp):
    nc = _get_nc()
    in_maps = _build_in_maps(x, gamma, beta, Wq, bq, Wk, bk, Wv, bv, Wp, bp)

    from concourse.bass_utils import run_bass_kernel_spmd

    res = run_bass_kernel_spmd(nc, in_maps, list(range(8)))

    B = 4
    out = np.empty((B, C, HW), np.float32)
    for core in range(8):
        b, half = divmod(core, 2)
        out[b, :, half * IHALF:(half + 1) * IHALF] = res.results[core]["yout"]
    return out.reshape(B, C, 64, 64)



# revision 8
# speedup vs baseline: 2.3786x; 2.3786x over previous
"""AttnBlock (GroupNorm -> single-head attention over 64x64 tokens -> proj -> residual)
for Trainium2, SPMD over 8 NeuronCores.

Sharding: core = batch(4) x query-half(2) (token order along j is permutation-
invariant for softmax attention and GroupNorm stats).

All heavy matmuls run in fp8e4m3 with DoubleRow perf mode (contract 256/instr
at 0.5 cycles/row): QKV projections, S^T = k^T q, O = vT e, the softmax
denominator (ones-matmul), and the output projection.

Scaling scheme (all powers of 2, exact):
  weights stored as 8*W^T in fp8; q,k,v carry x8; S_psum = 64*S_true
  exp: et = exp(S_psum * 1/(64*sqrt(C)) - ln16) = e_true/16  (fp8 range safe)
  l_psum = sum(et)/8 = l_true/128 ; lrb = recip = 128/l_true
  o_bf = opsum * lrb = 64*O_norm (fp8) ; proj psum = 512*(Wp O_norm)
  out = ps*(1/512) + (x + bp + Wp bv)

Bias folds: bk dropped exactly (softmax shift invariance); bv folded into
bp_eff = bp + Wp@bv host-side; bq added on the q PSUM->SBUF copy.

Softmax exp is staged: S psum tiles are copied (Pool/DVE) to a bf16 SBUF
buffer of 8 j-chunks, then ONE 4096-wide ACT exp produces fp8 et directly.

Layouts (SBUF, partition dim first):
  h8,k8: [128, 4cc, 4096] channel on partitions, tokens free (fp8)
  q8   : [128, 4cc, 2048]
  vt8  : [128jc, 32, 512] token chunk on partitions, channel free (fp8)
  S^T  : psum [128 j, 512 i]; et: [128 j, 8jc, 512 i] fp8
  O    : psum [128 c, 512 i] accumulated over 16 j-pairs via DoubleRow
"""

import math
import numpy as np
import ml_dtypes

import concourse.bass as bass
import concourse.mybir as mybir
import concourse.tile as tile
from concourse import library_config

P = 128
C = 512
NCC = C // P          # 4 channel chunks
HW = 4096             # tokens per batch image
IHALF = 2048          # query tokens per core
NBLK = IHALF // 512   # 4 i-blocks of 512
NJC = HW // P         # 32 j chunks of 128
NJT = HW // 512       # 8 j tiles of 512
GS = 16               # channels per group
EPS = 1e-6
WS = 8.0
EXP_SCALE = 1.0 / (64.0 * math.sqrt(C))
EXP_BIAS = -math.log(16.0)

F32 = mybir.dt.float32
BF16 = mybir.dt.bfloat16
F8 = mybir.dt.float8e4
BF = ml_dtypes.bfloat16
E4 = ml_dtypes.float8_e4m3

DR = mybir.MatmulPerfMode.DoubleRow
ALU = mybir.AluOpType
AF = mybir.ActivationFunctionType


def _split_excess_waits(nc):
    """walrus in this container accepts only ONE sync-wait per instruction;
    move extra waits onto same-engine NOPs placed immediately before."""
    for fn in nc.m.functions:
        for bb in fn.blocks:
            insts = list(bb.instructions)
            out = []
            changed = False
            for inst in insts:
                si = inst.sync_info
                if si is not None and len(si.on_wait) > 1:
                    waits = list(si.on_wait)
                    for k, w in enumerate(waits[:-1]):
                        nop = mybir.InstNoOp(
                            name=f"{inst.name}-ws{k}",
                            sync_info=mybir.SyncInfo(on_wait=[w], on_update=[]),
                            bass_nofuse=True,
                            engine=inst.engine,
                        )
                        out.append(nop)
                    inst.sync_info = mybir.SyncInfo(
                        on_wait=[waits[-1]], on_update=list(si.on_update)
                    )
                    changed = True
                out.append(inst)
            if changed:
                bb.instructions = out


def build_nc(split_waits=True):
    nc = bass.Bass()

    xbf_d = nc.declare_dram_parameter("x_bf", [C, HW], BF16, isOutput=False)
    xres_d = nc.declare_dram_parameter("x_res", [C, IHALF], F32, isOutput=False)
    wq8_d = nc.declare_dram_parameter("wq8", [C, C], F8, isOutput=False)
    wk8_d = nc.declare_dram_parameter("wk8", [C, C], F8, isOutput=False)
    wv8_d = nc.declare_dram_parameter("wv8", [C, C], F8, isOutput=False)
    wp8_d = nc.declare_dram_parameter("wp8", [C, C], F8, isOutput=False)
    bq8_d = nc.declare_dram_parameter("bq8_pc", [P, NCC], F32, isOutput=False)
    bpe_d = nc.declare_dram_parameter("bpe_pc", [P, NCC], F32, isOutput=False)
    gamma_d = nc.declare_dram_parameter("gamma_pc", [P, NCC], F32, isOutput=False)
    beta_d = nc.declare_dram_parameter("beta_pc", [P, NCC], F32, isOutput=False)
    indh_d = nc.declare_dram_parameter("indh", [P, P // GS], F32, isOutput=False)
    indt_d = nc.declare_dram_parameter("indt", [P, P // GS], F32, isOutput=False)
    bcast16_d = nc.declare_dram_parameter("bcast16", [P // GS, P], F32, isOutput=False)
    y_d = nc.declare_dram_parameter("yout", [C, IHALF], F32, isOutput=True)

    from contextlib import ExitStack

    with tile.TileContext(nc) as tc:
        with ExitStack() as stack:
            wpool = stack.enter_context(tc.tile_pool(name="w", bufs=1))
            cpool = stack.enter_context(tc.tile_pool(name="const", bufs=1))
            hpool = stack.enter_context(tc.tile_pool(name="hbuf", bufs=1))
            kpool = stack.enter_context(tc.tile_pool(name="kbuf", bufs=1))
            vpool = stack.enter_context(tc.tile_pool(name="vbuf", bufs=1))
            qpool = stack.enter_context(tc.tile_pool(name="qbuf", bufs=1))
            wq8 = wpool.tile([P, NCC, C], F8, tag="wq8")
            wk8 = wpool.tile([P, NCC, C], F8, tag="wk8")
            wv8 = wpool.tile([P, NCC, C], F8, tag="wv8")
            wp8 = wpool.tile([P, NCC, C], F8, tag="wp8")

            bq8_sb = cpool.tile([P, NCC], F32, tag="bq8")
            bpe_sb = cpool.tile([P, NCC], F32, tag="bpe")
            gamma_sb = cpool.tile([P, NCC], F32, tag="gamma")
            beta_sb = cpool.tile([P, NCC], F32, tag="beta")
            indh_sb = cpool.tile([P, P // GS], F32, tag="indh")
            indt_sb = cpool.tile([P, P // GS], F32, tag="indt")
            bcast16_sb = cpool.tile([P // GS, P], F32, tag="bcast16")
            eps_sb = cpool.tile([P // GS, 1], F32, tag="eps")
            ebias_sb = cpool.tile([P, 1], F32, tag="ebias")
            ones8_sb = cpool.tile([P, 2, 1], F8, tag="ones8")

            h8 = hpool.tile([P, NCC, HW], F8, tag="h8")
            k8 = kpool.tile([P, NCC, HW], F8, tag="k8")
            vt8 = vpool.tile([P, NJC, C], F8, tag="vt8")
            q8 = qpool.tile([P, NCC, IHALF], F8, tag="q8")

            # gpsimd custom-op library (partition_broadcast)
            nc.gpsimd.load_library(library_config.proxy)
            nc.vector.memset(eps_sb[:], EPS)
            nc.vector.memset(ebias_sb[:], EXP_BIAS)
            nc.vector.memset(ones8_sb[:], 0.125)

            # ====== phase 0: DMA in, GN stats on 3 engines, h8 = fp8(x*sc+sh) ======
            with ExitStack() as stack0:
                xpool = stack0.enter_context(tc.tile_pool(name="xbuf", bufs=1))
                gpool = stack0.enter_context(tc.tile_pool(name="gn", bufs=2))
                gppool = stack0.enter_context(tc.tile_pool(name="gnp", bufs=2, space="PSUM"))
                xb = xpool.tile([P, NCC, HW], BF16, tag="xb")
                half = HW // 2
                # one chunk per DMA queue; two halves each so stats can start early
                for ci, eng in ((0, nc.sync), (1, nc.gpsimd), (2, nc.scalar)):
                    eng.dma_start(out=xb[:, ci, :half], in_=xbf_d[ci * P:(ci + 1) * P, :half])
                    eng.dma_start(out=xb[:, ci, half:], in_=xbf_d[ci * P:(ci + 1) * P, half:])
                nc.sync.dma_start(out=xb[:, 3, :half], in_=xbf_d[3 * P:4 * P, :half])
                nc.scalar.dma_start(out=xb[:, 3, half:], in_=xbf_d[3 * P:4 * P, half:])
                # weights on sync queue (k first), consts on gpsimd queue
                nc.sync.dma_start(out=wk8[:], in_=wk8_d[:].rearrange("(cc p) o -> p cc o", p=P))
                nc.sync.dma_start(out=wq8[:], in_=wq8_d[:].rearrange("(cc p) o -> p cc o", p=P))
                nc.sync.dma_start(out=wv8[:], in_=wv8_d[:].rearrange("(cc p) o -> p cc o", p=P))
                nc.sync.dma_start(out=wp8[:], in_=wp8_d[:].rearrange("(cc p) o -> p cc o", p=P))
                for t, d in (
                    (indh_sb, indh_d), (indt_sb, indt_d), (gamma_sb, gamma_d),
                    (beta_sb, beta_d), (bq8_sb, bq8_d), (bpe_sb, bpe_d),
                    (bcast16_sb, bcast16_d),
                ):
                    nc.gpsimd.dma_start(out=t[:], in_=d[:])

                scale_sb = gpool.tile([P, NCC], F32, tag="scale")
                shift_sb = gpool.tile([P, NCC], F32, tag="shift")
                gpsum = gppool.tile([P // GS, 2 * NCC], F32, tag="gstat")

                for ci in range(NCC):
                    # DVE: bn_stats over tokens 0..2047 (4 blocks of 512)
                    stats = gpool.tile([P, 4, 6], F32, tag="stats")
                    for sg in range(4):
                        nc.vector.bn_stats(
                            out=stats[:, sg, :],
                            in_=xb[:, ci, sg * 512:(sg + 1) * 512],
                        )
                    mv = gpool.tile([P, 2], F32, tag="mv")
                    nc.vector.bn_aggr(out=mv[:], in_=stats[:])
                    # u = [mean, E[x^2]] over the DVE half
                    u = gpool.tile([P, 2], F32, tag="u")
                    nc.vector.tensor_copy(out=u[:, 0:1], in_=mv[:, 0:1])
                    nc.vector.tensor_tensor(u[:, 1:2], mv[:, 0:1], mv[:, 0:1], ALU.mult)
                    nc.vector.tensor_add(u[:, 1:2], u[:, 1:2], mv[:, 1:2])
                    # ACT: raw sums over tokens 2048..3071
                    s_act = gpool.tile([P, 2], F32, tag="sact")
                    scr = gpool.tile([P, 1024], BF16, tag="scr")
                    nc.scalar.activation(
                        out=scr[:], in_=xb[:, ci, 2048:3072],
                        func=AF.Copy, accum_out=s_act[:, 0:1],
                    )
                    nc.scalar.activation(
                        out=scr[:], in_=xb[:, ci, 2048:3072],
                        func=AF.Square, accum_out=s_act[:, 1:2],
                    )
                    # Pool: raw sums over tokens 3072..4095
                    s_pool = gpool.tile([P, 2], F32, tag="spool")
                    scr2 = gpool.tile([P, 1024], BF16, tag="scr2")
                    nc.gpsimd.scalar_tensor_tensor(
                        out=scr2[:], in0=xb[:, ci, 3072:4096], scalar=1.0,
                        in1=xb[:, ci, 3072:4096], op0=ALU.mult, op1=ALU.bypass,
                        accum_out=s_pool[:, 0:1],
                    )
                    nc.gpsimd.scalar_tensor_tensor(
                        out=scr2[:], in0=xb[:, ci, 3072:4096], scalar=1.0,
                        in1=xb[:, ci, 3072:4096], op0=ALU.mult, op1=ALU.mult,
                        accum_out=s_pool[:, 1:2],
                    )
                    # group-reduce: indh has 1/32 (mean-halves), indt has 1/(16*4096)
                    gsl = gpsum[:, ci * 2:(ci + 1) * 2]
                    nc.tensor.matmul(gsl, lhsT=indh_sb[:], rhs=u[:], start=True, stop=False)
                    nc.tensor.matmul(gsl, lhsT=indt_sb[:], rhs=s_act[:], start=False, stop=False)
                    nc.tensor.matmul(gsl, lhsT=indt_sb[:], rhs=s_pool[:], start=False, stop=True)

                    # group mean/rstd -> broadcast -> per-channel scale/shift
                    gmr = gpool.tile([P // GS, 2], F32, tag="gmr", name=f"gmr{ci}")
                    nc.vector.tensor_copy(out=gmr[:], in_=gsl)
                    mu = gmr[:, 0:1]
                    var = gmr[:, 1:2]
                    tmpv = gpool.tile([P // GS, 1], F32, tag="tmpv")
                    nc.vector.tensor_tensor(tmpv[:], mu, mu, ALU.mult)
                    nc.vector.tensor_tensor(var, var, tmpv[:], ALU.subtract)
                    nc.scalar.activation(
                        out=var, in_=var, func=AF.Sqrt, bias=eps_sb[:], scale=1.0
                    )
                    nc.vector.reciprocal(out=var, in_=var)
                    bpsum = gppool.tile([P, 2], F32, tag="bc")
                    nc.tensor.matmul(
                        bpsum[:], lhsT=bcast16_sb[:], rhs=gmr[:], start=True, stop=True
                    )
                    sc = scale_sb[:, ci:ci + 1]
                    sh = shift_sb[:, ci:ci + 1]
                    nc.vector.tensor_tensor(
                        sc, bpsum[:, 1:2], gamma_sb[:, ci:ci + 1], ALU.mult
                    )
                    nc.vector.tensor_tensor(sh, bpsum[:, 0:1], sc, ALU.mult)
                    nc.vector.tensor_tensor(
                        sh, beta_sb[:, ci:ci + 1], sh, ALU.subtract
                    )
                    # h8 = x*sc + sh in fp8; halves on Pool + (ACT for ci<3 else DVE)
                    nc.gpsimd.tensor_scalar(
                        out=h8[:, ci, :half], in0=xb[:, ci, :half],
                        scalar1=sc, scalar2=sh, op0=ALU.mult, op1=ALU.add,
                    )
                    if ci < 3:
                        nc.scalar.activation(
                            out=h8[:, ci, half:], in_=xb[:, ci, half:],
                            func=AF.Identity, bias=sh, scale=sc,
                        )
                    else:
                        nc.vector.tensor_scalar(
                            out=h8[:, ci, half:], in0=xb[:, ci, half:],
                            scalar1=sc, scalar2=sh, op0=ALU.mult, op1=ALU.add,
                        )


            # ====== phase 1: QKV projections (DoubleRow fp8) ======
            ncpy = [0]

            def cpy_engine():
                ncpy[0] += 1
                return nc.gpsimd if ncpy[0] % 2 == 0 else nc.vector

            def copy_to(eng, dst, src):
                if eng is nc.gpsimd:
                    eng.tensor_scalar(
                        out=dst, in0=src, scalar1=0.0, scalar2=None,
                        op0=ALU.add,
                    )
                else:
                    eng.tensor_copy(out=dst, in_=src)

            with tc.tile_pool(name="mmp", bufs=3, space="PSUM") as mmpool:

                def emit_k(jtp):
                    # k for j tiles (2*jtp, 2*jtp+1), all out chunks
                    for oc in range(NCC):
                        ps = mmpool.tile([P, 2, 512], F32, tag="mm")
                        for t in range(2):
                            jt = jtp * 2 + t
                            for g in range(2):
                                nc.tensor.matmul(
                                    ps[:, t, :],
                                    lhsT=wk8[:, 2 * g:2 * g + 2, oc * P:(oc + 1) * P],
                                    rhs=h8[:, 2 * g:2 * g + 2, jt * 512:(jt + 1) * 512],
                                    start=(g == 0), stop=(g == 1), perf_mode=DR,
                                )
                        copy_to(
                            cpy_engine(),
                            k8[:, oc, jtp * 1024:(jtp + 1) * 1024], ps[:, :, :],
                        )

                def emit_q(itp):
                    for oc in range(NCC):
                        ps = mmpool.tile([P, 2, 512], F32, tag="mm")
                        for t in range(2):
                            it = itp * 2 + t
                            for g in range(2):
                                nc.tensor.matmul(
                                    ps[:, t, :],
                                    lhsT=wq8[:, 2 * g:2 * g + 2, oc * P:(oc + 1) * P],
                                    rhs=h8[:, 2 * g:2 * g + 2, it * 512:(it + 1) * 512],
                                    start=(g == 0), stop=(g == 1), perf_mode=DR,
                                )
                        # copy + bq (x8) bias
                        eng = cpy_engine()
                        eng.tensor_scalar(
                            out=q8[:, oc, itp * 1024:(itp + 1) * 1024],
                            in0=ps[:, :, :], scalar1=bq8_sb[:, oc:oc + 1],
                            scalar2=None, op0=ALU.add,
                        )

                def emit_v(jcp):
                    # vT for j chunks (2*jcp, 2*jcp+1)
                    ps = mmpool.tile([P, 2, 512], F32, tag="mm")
                    for t in range(2):
                        jc = jcp * 2 + t
                        for g in range(2):
                            nc.tensor.matmul(
                                ps[:, t, :],
                                lhsT=h8[:, 2 * g:2 * g + 2, jc * P:(jc + 1) * P],
                                rhs=wv8[:, 2 * g:2 * g + 2, :],
                                start=(g == 0), stop=(g == 1), perf_mode=DR,
                            )
                    copy_to(cpy_engine(), vt8[:, jcp * 2:jcp * 2 + 2, :], ps[:, :, :])

                emit_k(0)
                emit_q(0)
                for jcp in range(0, 4):
                    emit_v(jcp)
                emit_k(1)
                for jcp in range(4, 8):
                    emit_v(jcp)
                emit_k(2)
                for jcp in range(8, 12):
                    emit_v(jcp)
                emit_k(3)
                for jcp in range(12, 16):
                    emit_v(jcp)
                emit_q(1)

            # ====== phase 2: attention + phase 3 projection ======
            with ExitStack() as stack1:
                stgpool = stack1.enter_context(tc.tile_pool(name="stg", bufs=2))
                etpool = stack1.enter_context(tc.tile_pool(name="et", bufs=2))
                obpool = stack1.enter_context(tc.tile_pool(name="ob", bufs=NBLK))
                lbpool = stack1.enter_context(tc.tile_pool(name="lb", bufs=2))
                lrbpool = stack1.enter_context(tc.tile_pool(name="lrb", bufs=2))
                stpool = stack1.enter_context(tc.tile_pool(name="stp", bufs=3, space="PSUM"))
                oapool = stack1.enter_context(tc.tile_pool(name="oap", bufs=1, space="PSUM"))
                lpool = stack1.enter_context(tc.tile_pool(name="lp", bufs=1, space="PSUM"))
                xrpool = stack1.enter_context(tc.tile_pool(name="xr", bufs=4))
                ospool = stack1.enter_context(tc.tile_pool(name="os", bufs=4))

                def attn_batch(ib, b, opsum, lpsum):
                    isl = slice(ib * 512, (ib + 1) * 512)
                    stage = stgpool.tile([P, 8, 512], BF16, tag="stage")
                    for g in range(8):
                        jc = b * 8 + g
                        st = stpool.tile([P, 512], F32, tag="st")
                        for gg in range(2):
                            nc.tensor.matmul(
                                st[:],
                                lhsT=k8[:, 2 * gg:2 * gg + 2, jc * P:(jc + 1) * P],
                                rhs=q8[:, 2 * gg:2 * gg + 2, isl],
                                start=(gg == 0), stop=(gg == 1), perf_mode=DR,
                            )
                        copy_to(cpy_engine(), stage[:, g, :], st[:])
                    et = etpool.tile([P, 8, 512], F8, tag="et")
                    nc.scalar.activation(
                        out=et[:], in_=stage[:], func=AF.Exp,
                        scale=EXP_SCALE, bias=ebias_sb[:],
                    )
                    for p in range(4):
                        pair = b * 4 + p
                        jc0 = pair * 2
                        first = pair == 0
                        last = pair == 15
                        for cc in range(NCC):
                            nc.tensor.matmul(
                                opsum[cc][:],
                                lhsT=vt8[:, jc0:jc0 + 2, cc * P:(cc + 1) * P],
                                rhs=et[:, 2 * p:2 * p + 2, :],
                                start=first, stop=last, perf_mode=DR,
                            )
                        nc.tensor.matmul(
                            lpsum[:],
                            lhsT=ones8_sb[:],
                            rhs=et[:, 2 * p:2 * p + 2, :],
                            start=first, stop=last, perf_mode=DR,
                        )

                def attn_tail(ib, opsum, lpsum):
                    linv = lbpool.tile([1, 512], F32, tag="linv")
                    nc.vector.reciprocal(out=linv[:], in_=lpsum[:])
                    lrb = lrbpool.tile([P, 512], F32, tag="lrb")
                    nc.gpsimd.partition_broadcast(lrb[:, :], linv[0:1, :], channels=P)
                    obf = obpool.tile([P, NCC, 512], F8, tag="obf", name=f"obf{ib}")
                    for cc in range(NCC):
                        eng = nc.vector if cc % 2 == 0 else nc.gpsimd
                        eng.tensor_tensor(obf[:, cc, :], opsum[cc][:], lrb[:, :], ALU.mult)
                    return obf

                o_bfs = []
                for ib in range(NBLK):
                    opsum = [
                        oapool.tile([P, 512], F32, tag=f"o{cc}", name=f"op{cc}")
                        for cc in range(NCC)
                    ]
                    lpsum = lpool.tile([1, 512], F32, tag="l")
                    for b in range(4):
                        attn_batch(ib, b, opsum, lpsum)
                    o_bfs.append(attn_tail(ib, opsum, lpsum))

                # ====== phase 3: out = (Wp @ O)/512 + (x + bp_eff) ======
                for ib in range(NBLK):
                    isl = slice(ib * 512, (ib + 1) * 512)
                    obf = o_bfs[ib]
                    for oc in range(NCC):
                        xr = xrpool.tile([P, 512], F32, tag="xr")
                        nc.gpsimd.dma_start(
                            out=xr[:], in_=xres_d[oc * P:(oc + 1) * P, isl]
                        )
                        nc.gpsimd.tensor_scalar(
                            out=xr[:], in0=xr[:], scalar1=bpe_sb[:, oc:oc + 1],
                            scalar2=None, op0=ALU.add,
                        )
                        ps = stpool.tile([P, 512], F32, tag="st")
                        for g in range(2):
                            nc.tensor.matmul(
                                ps[:],
                                lhsT=wp8[:, 2 * g:2 * g + 2, oc * P:(oc + 1) * P],
                                rhs=obf[:, 2 * g:2 * g + 2, :],
                                start=(g == 0), stop=(g == 1), perf_mode=DR,
                            )
                        ost = ospool.tile([P, 512], F32, tag="ost")
                        nc.vector.scalar_tensor_tensor(
                            out=ost[:], in0=ps[:], scalar=1.0 / 512.0,
                            in1=xr[:], op0=ALU.mult, op1=ALU.add,
                        )
                        nc.sync.dma_start(out=y_d[oc * P:(oc + 1) * P, isl], in_=ost[:])

    if split_waits:
        _split_excess_waits(nc)
    return nc


_NC = None


def _get_nc():
    global _NC
    if _NC is None:
        _NC = build_nc()
    return _NC


def _core0_feed(inputs):
    """Input map for core 0 (batch 0, first query half) — used by test harnesses."""
    maps = _build_in_maps(**inputs)
    return maps[0]


def _build_in_maps(x, gamma, beta, Wq, bq, Wk, bk, Wv, bv, Wp, bp):
    x = np.asarray(x, dtype=np.float32)
    B, c, H, W = x.shape
    assert (B, c, H, W) == (4, C, 64, 64)

    def pc(v):  # [C] -> [P, NCC]
        return np.ascontiguousarray(np.asarray(v, np.float32).reshape(NCC, P).T)

    indh = np.zeros((P, P // GS), np.float32)
    indh[np.arange(P), np.arange(P) // GS] = 1.0 / (GS * 2.0)
    indt = np.zeros((P, P // GS), np.float32)
    indt[np.arange(P), np.arange(P) // GS] = 1.0 / (GS * HW)
    bcast16 = np.zeros((P // GS, P), np.float32)
    bcast16[np.arange(P) // GS, np.arange(P)] = 1.0

    Wp32 = np.asarray(Wp, np.float32)
    bv32 = np.asarray(bv, np.float32)
    bp_eff = np.asarray(bp, np.float32) + Wp32 @ bv32

    def w8(wmat):
        return np.ascontiguousarray(
            np.asarray(wmat, np.float32).T * WS
        ).astype(E4)

    shared = {
        "wq8": w8(Wq), "wk8": w8(Wk), "wv8": w8(Wv), "wp8": w8(Wp),
        "bq8_pc": pc(np.asarray(bq, np.float32) * WS),
        "bpe_pc": pc(bp_eff),
        "gamma_pc": pc(gamma), "beta_pc": pc(beta),
        "indh": indh, "indt": indt, "bcast16": bcast16,
    }

    xf = x.reshape(B, C, HW)
    in_maps = []
    for core in range(8):
        b, halfsel = divmod(core, 2)
        xb = xf[b]
        if halfsel == 0:
            x_bc = xb
        else:
            x_bc = np.concatenate([xb[:, IHALF:], xb[:, :IHALF]], axis=1)
        x_bc = np.ascontiguousarray(x_bc)
        in_maps.append({
            "x_bf": x_bc.astype(BF),
            "x_res": np.ascontiguousarray(x_bc[:, :IHALF]),
            **shared,
        })
    return in_maps


def kernel(x, gamma, beta, Wq, bq, Wk, bk, Wv, bv, Wp, bp):
    nc = _get_nc()
    in_maps = _build_in_maps(x, gamma, beta, Wq, bq, Wk, bk, Wv, bv, Wp, bp)

    from concourse.bass_utils import run_bass_kernel_spmd

    res = run_bass_kernel_spmd(nc, in_maps, list(range(8)))

    B = 4
    out = np.empty((B, C, HW), np.float32)
    for core in range(8):
        b, halfsel = divmod(core, 2)
        out[b, :, halfsel * IHALF:(halfsel + 1) * IHALF] = res.results[core]["yout"]
    return out.reshape(B, C, 64, 64)


# revision 14
# speedup vs baseline: 2.4297x; 1.0215x over previous
"""AttnBlock (GroupNorm -> single-head attention over 64x64 tokens -> proj -> residual)
for Trainium2, SPMD over 8 NeuronCores.

Sharding: core = batch(4) x query-half(2) (token order along j is permutation-
invariant for softmax attention and GroupNorm stats).

All heavy matmuls run in fp8e4m3 with DoubleRow perf mode (contract 256/instr
at 0.5 cycles/row): QKV projections, S^T = k^T q, O = vT e, the softmax
denominator (ones-matmul), and the output projection.

Scaling scheme (all powers of 2, exact):
  weights stored as 8*W^T in fp8; q,k,v carry x8; S_psum = 64*S_true
  exp: et = exp(S_psum * 1/(64*sqrt(C)) - ln16) = e_true/16  (fp8 range safe)
  l_psum = sum(et)/8 = l_true/128 ; lrb = recip = 128/l_true
  o_bf = opsum * lrb = 64*O_norm (fp8) ; proj psum = 512*(Wp O_norm)
  out = ps*(1/512) + (x + bp + Wp bv)

Bias folds: bk dropped exactly (softmax shift invariance); bv folded into
bp_eff = bp + Wp@bv host-side; bq added on the q PSUM->SBUF copy.

Softmax exp is staged: S psum tiles are copied (Pool/DVE) to a bf16 SBUF
buffer of 8 j-chunks, then ONE 4096-wide ACT exp produces fp8 et directly.

Layouts (SBUF, partition dim first):
  h8,k8: [128, 4cc, 4096] channel on partitions, tokens free (fp8)
  q8   : [128, 4cc, 2048]
  vt8  : [128jc, 32, 512] token chunk on partitions, channel free (fp8)
  S^T  : psum [128 j, 512 i]; et: [128 j, 8jc, 512 i] fp8
  O    : psum [128 c, 512 i] accumulated over 16 j-pairs via DoubleRow
"""

import math
import numpy as np
import ml_dtypes

import concourse.bass as bass
import concourse.mybir as mybir
import concourse.tile as tile
from concourse import library_config

P = 128
C = 512
NCC = C // P          # 4 channel chunks
HW = 4096             # tokens per batch image
IHALF = 2048          # query tokens per core
NBLK = IHALF // 512   # 4 i-blocks of 512
NJC = HW // P         # 32 j chunks of 128
NJT = HW // 512       # 8 j tiles of 512
GS = 16               # channels per group
EPS = 1e-6
WS = 8.0
EXP_SCALE = 1.0 / (64.0 * math.sqrt(C))
EXP_BIAS = -math.log(16.0)

F32 = mybir.dt.float32
BF16 = mybir.dt.bfloat16
F8 = mybir.dt.float8e4
BF = ml_dtypes.bfloat16
E4 = ml_dtypes.float8_e4m3

DR = mybir.MatmulPerfMode.DoubleRow
ALU = mybir.AluOpType
AF = mybir.ActivationFunctionType


def _split_excess_waits(nc):
    """walrus in this container accepts only ONE sync-wait per instruction;
    move extra waits onto same-engine NOPs placed immediately before."""
    for fn in nc.m.functions:
        for bb in fn.blocks:
            insts = list(bb.instructions)
            out = []
            changed = False
            for inst in insts:
                si = inst.sync_info
                if si is not None and len(si.on_wait) > 1:
                    waits = list(si.on_wait)
                    for k, w in enumerate(waits[:-1]):
                        nop = mybir.InstNoOp(
                            name=f"{inst.name}-ws{k}",
                            sync_info=mybir.SyncInfo(on_wait=[w], on_update=[]),
                            bass_nofuse=True,
                            engine=inst.engine,
                        )
                        out.append(nop)
                    inst.sync_info = mybir.SyncInfo(
                        on_wait=[waits[-1]], on_update=list(si.on_update)
                    )
                    changed = True
                out.append(inst)
            if changed:
                bb.instructions = out


def build_nc(split_waits=True):
    nc = bass.Bass()

    xbf_d = nc.declare_dram_parameter("x_bf", [C, HW], BF16, isOutput=False)
    xres_d = nc.declare_dram_parameter("x_res", [C, IHALF], F32, isOutput=False)
    wq8_d = nc.declare_dram_parameter("wq8", [C, C], F8, isOutput=False)
    wk8_d = nc.declare_dram_parameter("wk8", [C, C], F8, isOutput=False)
    wv8_d = nc.declare_dram_parameter("wv8", [C, C], F8, isOutput=False)
    wp8_d = nc.declare_dram_parameter("wp8", [C, C], F8, isOutput=False)
    bq8_d = nc.declare_dram_parameter("bq8_pc", [P, NCC], F32, isOutput=False)
    bpe_d = nc.declare_dram_parameter("bpe_pc", [P, NCC], F32, isOutput=False)
    gamma_d = nc.declare_dram_parameter("gamma_pc", [P, NCC], F32, isOutput=False)
    beta_d = nc.declare_dram_parameter("beta_pc", [P, NCC], F32, isOutput=False)
    indh_d = nc.declare_dram_parameter("indh", [P, P // GS], F32, isOutput=False)
    indt_d = nc.declare_dram_parameter("indt", [P, P // GS], F32, isOutput=False)
    bcast16_d = nc.declare_dram_parameter("bcast16", [P // GS, P], F32, isOutput=False)
    y_d = nc.declare_dram_parameter("yout", [C, IHALF], F32, isOutput=True)

    from contextlib import ExitStack

    with tile.TileContext(nc) as tc:
        with ExitStack() as stack:
            wpool = stack.enter_context(tc.tile_pool(name="w", bufs=1))
            cpool = stack.enter_context(tc.tile_pool(name="const", bufs=1))
            hpool = stack.enter_context(tc.tile_pool(name="hbuf", bufs=1))
            kpool = stack.enter_context(tc.tile_pool(name="kbuf", bufs=1))
            vpool = stack.enter_context(tc.tile_pool(name="vbuf", bufs=1))
            qpool = stack.enter_context(tc.tile_pool(name="qbuf", bufs=1))
            wq8 = wpool.tile([P, NCC, C], F8, tag="wq8")
            wk8 = wpool.tile([P, NCC, C], F8, tag="wk8")
            wv8 = wpool.tile([P, NCC, C], F8, tag="wv8")
            wp8 = wpool.tile([P, NCC, C], F8, tag="wp8")

            bq8_sb = cpool.tile([P, NCC], F32, tag="bq8")
            bpe_sb = cpool.tile([P, NCC], F32, tag="bpe")
            gamma_sb = cpool.tile([P, NCC], F32, tag="gamma")
            beta_sb = cpool.tile([P, NCC], F32, tag="beta")
            indh_sb = cpool.tile([P, P // GS], F32, tag="indh")
            indt_sb = cpool.tile([P, P // GS], F32, tag="indt")
            bcast16_sb = cpool.tile([P // GS, P], F32, tag="bcast16")
            eps_sb = cpool.tile([P // GS, 1], F32, tag="eps")
            ebias_sb = cpool.tile([P, 1], F32, tag="ebias")
            ones8_sb = cpool.tile([P, 2, 1], F8, tag="ones8")

            h8 = hpool.tile([P, NCC, HW], F8, tag="h8")
            k8 = kpool.tile([P, NCC, HW], F8, tag="k8")
            vt8 = vpool.tile([P, NJC, C], F8, tag="vt8")
            q8 = qpool.tile([P, NCC, IHALF], F8, tag="q8")

            # gpsimd custom-op library (partition_broadcast)
            nc.gpsimd.load_library(library_config.proxy)
            nc.vector.memset(eps_sb[:], EPS)
            nc.vector.memset(ebias_sb[:], EXP_BIAS)
            nc.vector.memset(ones8_sb[:], 0.125)

            # ====== phase 0: DMA in, GN stats on 3 engines, h8 = fp8(x*sc+sh) ======
            with ExitStack() as stack0:
                xpool = stack0.enter_context(tc.tile_pool(name="xbuf", bufs=1))
                gpool = stack0.enter_context(tc.tile_pool(name="gn", bufs=2))
                gppool = stack0.enter_context(tc.tile_pool(name="gnp", bufs=2, space="PSUM"))
                xb = xpool.tile([P, NCC, HW], BF16, tag="xb")
                half = HW // 2
                # one chunk per DMA queue; two halves each so stats can start early
                for ci, eng in ((0, nc.sync), (1, nc.gpsimd), (2, nc.scalar)):
                    eng.dma_start(out=xb[:, ci, :half], in_=xbf_d[ci * P:(ci + 1) * P, :half])
                    eng.dma_start(out=xb[:, ci, half:], in_=xbf_d[ci * P:(ci + 1) * P, half:])
                nc.sync.dma_start(out=xb[:, 3, :half], in_=xbf_d[3 * P:4 * P, :half])
                nc.scalar.dma_start(out=xb[:, 3, half:], in_=xbf_d[3 * P:4 * P, half:])
                # weights on sync queue (k first), consts on gpsimd queue
                nc.sync.dma_start(out=wk8[:], in_=wk8_d[:].rearrange("(cc p) o -> p cc o", p=P))
                nc.sync.dma_start(out=wq8[:], in_=wq8_d[:].rearrange("(cc p) o -> p cc o", p=P))
                nc.sync.dma_start(out=wv8[:], in_=wv8_d[:].rearrange("(cc p) o -> p cc o", p=P))
                nc.sync.dma_start(out=wp8[:], in_=wp8_d[:].rearrange("(cc p) o -> p cc o", p=P))
                for t, d in (
                    (indh_sb, indh_d), (indt_sb, indt_d), (gamma_sb, gamma_d),
                    (beta_sb, beta_d), (bq8_sb, bq8_d), (bpe_sb, bpe_d),
                    (bcast16_sb, bcast16_d),
                ):
                    nc.gpsimd.dma_start(out=t[:], in_=d[:])

                scale_sb = gpool.tile([P, NCC], F32, tag="scale")
                shift_sb = gpool.tile([P, NCC], F32, tag="shift")
                gpsum = gppool.tile([P // GS, 2 * NCC], F32, tag="gstat")

                for ci in range(NCC):
                    # DVE: bn_stats over tokens 0..2047 (4 blocks of 512)
                    stats = gpool.tile([P, 4, 6], F32, tag="stats")
                    for sg in range(4):
                        nc.vector.bn_stats(
                            out=stats[:, sg, :],
                            in_=xb[:, ci, sg * 512:(sg + 1) * 512],
                        )
                    mv = gpool.tile([P, 2], F32, tag="mv")
                    nc.vector.bn_aggr(out=mv[:], in_=stats[:])
                    # u = [mean, E[x^2]] over the DVE half
                    u = gpool.tile([P, 2], F32, tag="u")
                    nc.vector.tensor_copy(out=u[:, 0:1], in_=mv[:, 0:1])
                    nc.vector.tensor_tensor(u[:, 1:2], mv[:, 0:1], mv[:, 0:1], ALU.mult)
                    nc.vector.tensor_add(u[:, 1:2], u[:, 1:2], mv[:, 1:2])
                    # ACT: raw sums over tokens 2048..3071
                    s_act = gpool.tile([P, 2], F32, tag="sact")
                    scr = gpool.tile([P, 1024], BF16, tag="scr")
                    nc.scalar.activation(
                        out=scr[:], in_=xb[:, ci, 2048:3072],
                        func=AF.Copy, accum_out=s_act[:, 0:1],
                    )
                    nc.scalar.activation(
                        out=scr[:], in_=xb[:, ci, 2048:3072],
                        func=AF.Square, accum_out=s_act[:, 1:2],
                    )
                    # Pool: raw sums over tokens 3072..4095
                    s_pool = gpool.tile([P, 2], F32, tag="spool")
                    scr2 = gpool.tile([P, 1024], BF16, tag="scr2")
                    nc.gpsimd.scalar_tensor_tensor(
                        out=scr2[:], in0=xb[:, ci, 3072:4096], scalar=1.0,
                        in1=xb[:, ci, 3072:4096], op0=ALU.mult, op1=ALU.bypass,
                        accum_out=s_pool[:, 0:1],
                    )
                    nc.gpsimd.scalar_tensor_tensor(
                        out=scr2[:], in0=xb[:, ci, 3072:4096], scalar=1.0,
                        in1=xb[:, ci, 3072:4096], op0=ALU.mult, op1=ALU.mult,
                        accum_out=s_pool[:, 1:2],
                    )
                    # group-reduce: indh has 1/32 (mean-halves), indt has 1/(16*4096)
                    gsl = gpsum[:, ci * 2:(ci + 1) * 2]
                    nc.tensor.matmul(gsl, lhsT=indh_sb[:], rhs=u[:], start=True, stop=False)
                    nc.tensor.matmul(gsl, lhsT=indt_sb[:], rhs=s_act[:], start=False, stop=False)
                    nc.tensor.matmul(gsl, lhsT=indt_sb[:], rhs=s_pool[:], start=False, stop=True)

                    # group mean/rstd -> broadcast -> per-channel scale/shift
                    gmr = gpool.tile([P // GS, 2], F32, tag="gmr", name=f"gmr{ci}")
                    nc.vector.tensor_copy(out=gmr[:], in_=gsl)
                    mu = gmr[:, 0:1]
                    var = gmr[:, 1:2]
                    tmpv = gpool.tile([P // GS, 1], F32, tag="tmpv")
                    nc.vector.tensor_tensor(tmpv[:], mu, mu, ALU.mult)
                    nc.vector.tensor_tensor(var, var, tmpv[:], ALU.subtract)
                    nc.scalar.activation(
                        out=var, in_=var, func=AF.Sqrt, bias=eps_sb[:], scale=1.0
                    )
                    nc.vector.reciprocal(out=var, in_=var)
                    bpsum = gppool.tile([P, 2], F32, tag="bc")
                    nc.tensor.matmul(
                        bpsum[:], lhsT=bcast16_sb[:], rhs=gmr[:], start=True, stop=True
                    )
                    sc = scale_sb[:, ci:ci + 1]
                    sh = shift_sb[:, ci:ci + 1]
                    nc.vector.tensor_tensor(
                        sc, bpsum[:, 1:2], gamma_sb[:, ci:ci + 1], ALU.mult
                    )
                    nc.vector.tensor_tensor(sh, bpsum[:, 0:1], sc, ALU.mult)
                    nc.vector.tensor_tensor(
                        sh, beta_sb[:, ci:ci + 1], sh, ALU.subtract
                    )
                    # h8 = x*sc + sh in fp8; halves on Pool + (ACT for ci<2 else DVE)
                    nc.gpsimd.tensor_scalar(
                        out=h8[:, ci, :half], in0=xb[:, ci, :half],
                        scalar1=sc, scalar2=sh, op0=ALU.mult, op1=ALU.add,
                    )
                    if ci < 2:
                        nc.scalar.activation(
                            out=h8[:, ci, half:], in_=xb[:, ci, half:],
                            func=AF.Identity, bias=sh, scale=sc,
                        )
                    else:
                        nc.vector.tensor_scalar(
                            out=h8[:, ci, half:], in0=xb[:, ci, half:],
                            scalar1=sc, scalar2=sh, op0=ALU.mult, op1=ALU.add,
                        )


            # ====== phase 1: QKV projections (DoubleRow fp8) ======
            # PSUM->SBUF fp8 conversion copies rotate over Pool/DVE/ACT (ACT is
            # idle during this phase); bias is folded into the copy where needed.
            ncpy = [0]

            def cpy3_engine():
                ncpy[0] += 1
                return (nc.gpsimd, nc.vector, nc.scalar)[ncpy[0] % 3]

            def copy_to(eng, dst, src, bias=None):
                if eng is nc.scalar:
                    if bias is None:
                        eng.activation(out=dst, in_=src, func=AF.Copy)
                    else:
                        eng.activation(out=dst, in_=src, func=AF.Identity, bias=bias)
                elif eng is nc.gpsimd:
                    eng.tensor_scalar(
                        out=dst, in0=src,
                        scalar1=(0.0 if bias is None else bias), scalar2=None,
                        op0=ALU.add,
                    )
                else:
                    if bias is None:
                        eng.tensor_copy(out=dst, in_=src)
                    else:
                        eng.tensor_scalar(
                            out=dst, in0=src, scalar1=bias, scalar2=None,
                            op0=ALU.add,
                        )

            with tc.tile_pool(name="mmp", bufs=4, space="PSUM") as mmpool:

                def emit_k(jtp):
                    # k for j tiles (2*jtp, 2*jtp+1), all out chunks
                    for oc in range(NCC):
                        ps = mmpool.tile([P, 2, 512], F32, tag="mm")
                        for t in range(2):
                            jt = jtp * 2 + t
                            for g in range(2):
                                nc.tensor.matmul(
                                    ps[:, t, :],
                                    lhsT=wk8[:, 2 * g:2 * g + 2, oc * P:(oc + 1) * P],
                                    rhs=h8[:, 2 * g:2 * g + 2, jt * 512:(jt + 1) * 512],
                                    start=(g == 0), stop=(g == 1), perf_mode=DR,
                                )
                        copy_to(
                            cpy3_engine(),
                            k8[:, oc, jtp * 1024:(jtp + 1) * 1024], ps[:, :, :],
                        )

                def emit_q(itp):
                    for oc in range(NCC):
                        ps = mmpool.tile([P, 2, 512], F32, tag="mm")
                        for t in range(2):
                            it = itp * 2 + t
                            for g in range(2):
                                nc.tensor.matmul(
                                    ps[:, t, :],
                                    lhsT=wq8[:, 2 * g:2 * g + 2, oc * P:(oc + 1) * P],
                                    rhs=h8[:, 2 * g:2 * g + 2, it * 512:(it + 1) * 512],
                                    start=(g == 0), stop=(g == 1), perf_mode=DR,
                                )
                        # copy + bq (x8) bias
                        copy_to(
                            cpy3_engine(),
                            q8[:, oc, itp * 1024:(itp + 1) * 1024], ps[:, :, :],
                            bias=bq8_sb[:, oc:oc + 1],
                        )

                def emit_v(jcp):
                    # vT for j chunks (2*jcp, 2*jcp+1)
                    ps = mmpool.tile([P, 2, 512], F32, tag="mm")
                    for t in range(2):
                        jc = jcp * 2 + t
                        for g in range(2):
                            nc.tensor.matmul(
                                ps[:, t, :],
                                lhsT=h8[:, 2 * g:2 * g + 2, jc * P:(jc + 1) * P],
                                rhs=wv8[:, 2 * g:2 * g + 2, :],
                                start=(g == 0), stop=(g == 1), perf_mode=DR,
                            )
                    copy_to(cpy3_engine(), vt8[:, jcp * 2:jcp * 2 + 2, :], ps[:, :, :])

                emit_k(0)
                emit_q(0)
                for jcp in range(0, 4):
                    emit_v(jcp)
                emit_k(1)
                for jcp in range(4, 8):
                    emit_v(jcp)
                emit_k(2)
                for jcp in range(8, 12):
                    emit_v(jcp)
                emit_k(3)
                for jcp in range(12, 16):
                    emit_v(jcp)
                emit_q(1)

            # ====== phase 2: attention + phase 3 projection ======
            with ExitStack() as stack1:
                stgpool = stack1.enter_context(tc.tile_pool(name="stg", bufs=3))
                etpool = stack1.enter_context(tc.tile_pool(name="et", bufs=3))
                obpool = stack1.enter_context(tc.tile_pool(name="ob", bufs=NBLK))
                lbpool = stack1.enter_context(tc.tile_pool(name="lb", bufs=2))
                lrbpool = stack1.enter_context(tc.tile_pool(name="lrb", bufs=2))
                stpool = stack1.enter_context(tc.tile_pool(name="stp", bufs=3, space="PSUM"))
                oapool = stack1.enter_context(tc.tile_pool(name="oap", bufs=1, space="PSUM"))
                lpool = stack1.enter_context(tc.tile_pool(name="lp", bufs=1, space="PSUM"))
                xrpool = stack1.enter_context(tc.tile_pool(name="xr", bufs=4))
                ospool = stack1.enter_context(tc.tile_pool(name="os", bufs=4))

                def attn_batch(ib, b, opsum, lpsum):
                    isl = slice(ib * 512, (ib + 1) * 512)
                    stage = stgpool.tile([P, 8, 512], BF16, tag="stage")
                    npool = 5 if b % 2 == 0 else 4
                    for g in range(8):
                        jc = b * 8 + g
                        st = stpool.tile([P, 512], F32, tag="st")
                        for gg in range(2):
                            nc.tensor.matmul(
                                st[:],
                                lhsT=k8[:, 2 * gg:2 * gg + 2, jc * P:(jc + 1) * P],
                                rhs=q8[:, 2 * gg:2 * gg + 2, isl],
                                start=(gg == 0), stop=(gg == 1), perf_mode=DR,
                            )
                        copy_to(nc.gpsimd if g < npool else nc.vector,
                                stage[:, g, :], st[:])
                    et = etpool.tile([P, 8, 512], F8, tag="et")
                    nc.scalar.activation(
                        out=et[:], in_=stage[:], func=AF.Exp,
                        scale=EXP_SCALE, bias=ebias_sb[:],
                    )
                    for p in range(4):
                        pair = b * 4 + p
                        jc0 = pair * 2
                        first = pair == 0
                        last = pair == 15
                        for cc in range(NCC):
                            nc.tensor.matmul(
                                opsum[cc][:],
                                lhsT=vt8[:, jc0:jc0 + 2, cc * P:(cc + 1) * P],
                                rhs=et[:, 2 * p:2 * p + 2, :],
                                start=first, stop=last, perf_mode=DR,
                            )
                        nc.tensor.matmul(
                            lpsum[:],
                            lhsT=ones8_sb[:],
                            rhs=et[:, 2 * p:2 * p + 2, :],
                            start=first, stop=last, perf_mode=DR,
                        )

                def attn_tail(ib, opsum, lpsum):
                    linv = lbpool.tile([1, 512], F32, tag="linv")
                    nc.vector.reciprocal(out=linv[:], in_=lpsum[:])
                    lrb = lrbpool.tile([P, 512], F32, tag="lrb")
                    nc.gpsimd.partition_broadcast(lrb[:, :], linv[0:1, :], channels=P)
                    obf = obpool.tile([P, NCC, 512], F8, tag="obf", name=f"obf{ib}")
                    for cc in range(NCC):
                        eng = nc.vector if cc % 2 == 0 else nc.gpsimd
                        eng.tensor_tensor(obf[:, cc, :], opsum[cc][:], lrb[:, :], ALU.mult)
                    return obf

                def emit_proj(ib, obf):
                    # phase 3 for one i-block: out = (Wp @ O)/512 + (x + bp_eff)
                    isl = slice(ib * 512, (ib + 1) * 512)
                    for oc in range(NCC):
                        xr = xrpool.tile([P, 512], F32, tag="xr")
                        nc.sync.dma_start(
                            out=xr[:], in_=xres_d[oc * P:(oc + 1) * P, isl]
                        )
                        nc.gpsimd.tensor_scalar(
                            out=xr[:], in0=xr[:], scalar1=bpe_sb[:, oc:oc + 1],
                            scalar2=None, op0=ALU.add,
                        )
                        ps = stpool.tile([P, 512], F32, tag="st")
                        for g in range(2):
                            nc.tensor.matmul(
                                ps[:],
                                lhsT=wp8[:, 2 * g:2 * g + 2, oc * P:(oc + 1) * P],
                                rhs=obf[:, 2 * g:2 * g + 2, :],
                                start=(g == 0), stop=(g == 1), perf_mode=DR,
                            )
                        ost = ospool.tile([P, 512], F32, tag="ost")
                        nc.vector.scalar_tensor_tensor(
                            out=ost[:], in0=ps[:], scalar=1.0 / 512.0,
                            in1=xr[:], op0=ALU.mult, op1=ALU.add,
                        )
                        nc.sync.dma_start(out=y_d[oc * P:(oc + 1) * P, isl], in_=ost[:])

                o_bfs = []
                for ib in range(NBLK):
                    opsum = [
                        oapool.tile([P, 512], F32, tag=f"o{cc}", name=f"op{cc}")
                        for cc in range(NCC)
                    ]
                    lpsum = lpool.tile([1, 512], F32, tag="l")
                    for b in range(4):
                        attn_batch(ib, b, opsum, lpsum)
                        # interleave previous block's projection behind batch 1
                        if b == 1 and ib > 0:
                            emit_proj(ib - 1, o_bfs[ib - 1])
                    o_bfs.append(attn_tail(ib, opsum, lpsum))
                emit_proj(NBLK - 1, o_bfs[NBLK - 1])

    if split_waits:
        _split_excess_waits(nc)
    return nc


_NC = None


def _get_nc():
    global _NC
    if _NC is None:
        _NC = build_nc()
    return _NC


def _core0_feed(inputs):
    """Input map for core 0 (batch 0, first query half) — used by test harnesses."""
    maps = _build_in_maps(**inputs)
    return maps[0]


def _build_in_maps(x, gamma, beta, Wq, bq, Wk, bk, Wv, bv, Wp, bp):
    x = np.asarray(x, dtype=np.float32)
    B, c, H, W = x.shape
    assert (B, c, H, W) == (4, C, 64, 64)

    def pc(v):  # [C] -> [P, NCC]
        return np.ascontiguousarray(np.asarray(v, np.float32).reshape(NCC, P).T)

    indh = np.zeros((P, P // GS), np.float32)
    indh[np.arange(P), np.arange(P) // GS] = 1.0 / (GS * 2.0)
    indt = np.zeros((P, P // GS), np.float32)
    indt[np.arange(P), np.arange(P) // GS] = 1.0 / (GS * HW)
    bcast16 = np.zeros((P // GS, P), np.float32)
    bcast16[np.arange(P) // GS, np.arange(P)] = 1.0

    Wp32 = np.asarray(Wp, np.float32)
    bv32 = np.asarray(bv, np.float32)
    bp_eff = np.asarray(bp, np.float32) + Wp32 @ bv32

    def w8(wmat):
        return np.ascontiguousarray(
            np.asarray(wmat, np.float32).T * WS
        ).astype(E4)

    shared = {
        "wq8": w8(Wq), "wk8": w8(Wk), "wv8": w8(Wv), "wp8": w8(Wp),
        "bq8_pc": pc(np.asarray(bq, np.float32) * WS),
        "bpe_pc": pc(bp_eff),
        "gamma_pc": pc(gamma), "beta_pc": pc(beta),
        "indh": indh, "indt": indt, "bcast16": bcast16,
    }

    xf = x.reshape(B, C, HW)
    in_maps = []
    for core in range(8):
        b, halfsel = divmod(core, 2)
        xb = xf[b]
        if halfsel == 0:
            x_bc = xb
        else:
            x_bc = np.concatenate([xb[:, IHALF:], xb[:, :IHALF]], axis=1)
        x_bc = np.ascontiguousarray(x_bc)
        in_maps.append({
            "x_bf": x_bc.astype(BF),
            "x_res": np.ascontiguousarray(x_bc[:, :IHALF]),
            **shared,
        })
    return in_maps


def kernel(x, gamma, beta, Wq, bq, Wk, bk, Wv, bv, Wp, bp):
    nc = _get_nc()
    in_maps = _build_in_maps(x, gamma, beta, Wq, bq, Wk, bk, Wv, bv, Wp, bp)

    from concourse.bass_utils import run_bass_kernel_spmd

    res = run_bass_kernel_spmd(nc, in_maps, list(range(8)))

    B = 4
    out = np.empty((B, C, HW), np.float32)
    for core in range(8):
        b, halfsel = divmod(core, 2)
        out[b, :, halfsel * IHALF:(halfsel + 1) * IHALF] = res.results[core]["yout"]
    return out.reshape(B, C, 64, 64)


# revision 15
# speedup vs baseline: 2.4556x; 1.0107x over previous
"""AttnBlock (GroupNorm -> single-head attention over 64x64 tokens -> proj -> residual)
for Trainium2, SPMD over 8 NeuronCores.

Sharding: core = batch(4) x query-half(2) (token order along j is permutation-
invariant for softmax attention and GroupNorm stats).

All heavy matmuls run in fp8e4m3 with DoubleRow perf mode (contract 256/instr
at 0.5 cycles/row): QKV projections, S^T = k^T q, O = vT e, the softmax
denominator (ones-matmul), and the output projection.

Scaling scheme (all powers of 2, exact):
  weights stored as 8*W^T in fp8; q,k,v carry x8; S_psum = 64*S_true
  exp: et = exp(S_psum * 1/(64*sqrt(C)) - ln16) = e_true/16  (fp8 range safe)
  l_psum = sum(et)/8 = l_true/128 ; lrb = recip = 128/l_true
  o_bf = opsum * lrb = 64*O_norm (fp8) ; proj psum = 512*(Wp O_norm)
  out = ps*(1/512) + (x + bp + Wp bv)

Bias folds: bk dropped exactly (softmax shift invariance); bv folded into
bp_eff = bp + Wp@bv host-side; bq added on the q PSUM->SBUF copy.

Softmax exp is staged: S psum tiles are copied (Pool/DVE) to a bf16 SBUF
buffer of 8 j-chunks, then ONE 4096-wide ACT exp produces fp8 et directly.

Layouts (SBUF, partition dim first):
  h8,k8: [128, 4cc, 4096] channel on partitions, tokens free (fp8)
  q8   : [128, 4cc, 2048]
  vt8  : [128jc, 32, 512] token chunk on partitions, channel free (fp8)
  S^T  : psum [128 j, 512 i]; et: [128 j, 8jc, 512 i] fp8
  O    : psum [128 c, 512 i] accumulated over 16 j-pairs via DoubleRow
"""

import math
import numpy as np
import ml_dtypes

import concourse.bass as bass
import concourse.mybir as mybir
import concourse.tile as tile
from concourse import library_config

P = 128
C = 512
NCC = C // P          # 4 channel chunks
HW = 4096             # tokens per batch image
IHALF = 2048          # query tokens per core
NBLK = IHALF // 512   # 4 i-blocks of 512
NJC = HW // P         # 32 j chunks of 128
NJT = HW // 512       # 8 j tiles of 512
GS = 16               # channels per group
EPS = 1e-6
WS = 8.0
EXP_SCALE = 1.0 / (64.0 * math.sqrt(C))
EXP_BIAS = -math.log(16.0)

F32 = mybir.dt.float32
BF16 = mybir.dt.bfloat16
F8 = mybir.dt.float8e4
BF = ml_dtypes.bfloat16
E4 = ml_dtypes.float8_e4m3

DR = mybir.MatmulPerfMode.DoubleRow
ALU = mybir.AluOpType
AF = mybir.ActivationFunctionType


def _split_excess_waits(nc):
    """walrus in this container accepts only ONE sync-wait per instruction;
    move extra waits onto same-engine NOPs placed immediately before."""
    for fn in nc.m.functions:
        for bb in fn.blocks:
            insts = list(bb.instructions)
            out = []
            changed = False
            for inst in insts:
                si = inst.sync_info
                if si is not None and len(si.on_wait) > 1:
                    waits = list(si.on_wait)
                    for k, w in enumerate(waits[:-1]):
                        nop = mybir.InstNoOp(
                            name=f"{inst.name}-ws{k}",
                            sync_info=mybir.SyncInfo(on_wait=[w], on_update=[]),
                            bass_nofuse=True,
                            engine=inst.engine,
                        )
                        out.append(nop)
                    inst.sync_info = mybir.SyncInfo(
                        on_wait=[waits[-1]], on_update=list(si.on_update)
                    )
                    changed = True
                out.append(inst)
            if changed:
                bb.instructions = out


def build_nc(split_waits=True):
    nc = bass.Bass()

    xbf_d = nc.declare_dram_parameter("x_bf", [C, HW], BF16, isOutput=False)
    xres_d = nc.declare_dram_parameter("x_res", [C, IHALF], F32, isOutput=False)
    wq8_d = nc.declare_dram_parameter("wq8", [C, C], F8, isOutput=False)
    wk8_d = nc.declare_dram_parameter("wk8", [C, C], F8, isOutput=False)
    wv8_d = nc.declare_dram_parameter("wv8", [C, C], F8, isOutput=False)
    wp8_d = nc.declare_dram_parameter("wp8", [C, C], F8, isOutput=False)
    bq8_d = nc.declare_dram_parameter("bq8_pc", [P, NCC], F32, isOutput=False)
    bpe_d = nc.declare_dram_parameter("bpe_pc", [P, NCC], F32, isOutput=False)
    gamma_d = nc.declare_dram_parameter("gamma_pc", [P, NCC], F32, isOutput=False)
    beta_d = nc.declare_dram_parameter("beta_pc", [P, NCC], F32, isOutput=False)
    indh_d = nc.declare_dram_parameter("indh", [P, P // GS], F32, isOutput=False)
    indt_d = nc.declare_dram_parameter("indt", [P, P // GS], F32, isOutput=False)
    bcast16_d = nc.declare_dram_parameter("bcast16", [P // GS, P], F32, isOutput=False)
    y_d = nc.declare_dram_parameter("yout", [C, IHALF], F32, isOutput=True)

    from contextlib import ExitStack

    with tile.TileContext(nc) as tc:
        with ExitStack() as stack:
            wpool = stack.enter_context(tc.tile_pool(name="w", bufs=1))
            cpool = stack.enter_context(tc.tile_pool(name="const", bufs=1))
            hpool = stack.enter_context(tc.tile_pool(name="hbuf", bufs=1))
            kpool = stack.enter_context(tc.tile_pool(name="kbuf", bufs=1))
            vpool = stack.enter_context(tc.tile_pool(name="vbuf", bufs=1))
            qpool = stack.enter_context(tc.tile_pool(name="qbuf", bufs=1))
            wq8 = wpool.tile([P, NCC, C], F8, tag="wq8")
            wk8 = wpool.tile([P, NCC, C], F8, tag="wk8")
            wv8 = wpool.tile([P, NCC, C], F8, tag="wv8")
            wp8 = wpool.tile([P, NCC, C], F8, tag="wp8")

            bq8_sb = cpool.tile([P, NCC], F32, tag="bq8")
            bpe_sb = cpool.tile([P, NCC], F32, tag="bpe")
            gamma_sb = cpool.tile([P, NCC], F32, tag="gamma")
            beta_sb = cpool.tile([P, NCC], F32, tag="beta")
            indh_sb = cpool.tile([P, P // GS], F32, tag="indh")
            indt_sb = cpool.tile([P, P // GS], F32, tag="indt")
            bcast16_sb = cpool.tile([P // GS, P], F32, tag="bcast16")
            eps_sb = cpool.tile([P // GS, 1], F32, tag="eps")
            ebias_sb = cpool.tile([P, 1], F32, tag="ebias")
            ones8_sb = cpool.tile([P, 2, 1], F8, tag="ones8")

            h8 = hpool.tile([P, NCC, HW], F8, tag="h8")
            k8 = kpool.tile([P, NCC, HW], F8, tag="k8")
            vt8 = vpool.tile([P, NJC, C], F8, tag="vt8")
            q8 = qpool.tile([P, NCC, IHALF], F8, tag="q8")

            # gpsimd custom-op library (partition_broadcast)
            nc.gpsimd.load_library(library_config.proxy)
            nc.vector.memset(eps_sb[:], EPS)
            nc.vector.memset(ebias_sb[:], EXP_BIAS)
            nc.vector.memset(ones8_sb[:], 1.0)

            # ====== phase 0: DMA in, GN stats on 3 engines, h8 = fp8(x*sc+sh) ======
            with ExitStack() as stack0:
                xpool = stack0.enter_context(tc.tile_pool(name="xbuf", bufs=1))
                gpool = stack0.enter_context(tc.tile_pool(name="gn", bufs=2))
                gppool = stack0.enter_context(tc.tile_pool(name="gnp", bufs=2, space="PSUM"))
                xb = xpool.tile([P, NCC, HW], BF16, tag="xb")
                half = HW // 2
                # one chunk per DMA queue; two halves each so stats can start early
                for ci, eng in ((0, nc.sync), (1, nc.gpsimd), (2, nc.scalar)):
                    eng.dma_start(out=xb[:, ci, :half], in_=xbf_d[ci * P:(ci + 1) * P, :half])
                    eng.dma_start(out=xb[:, ci, half:], in_=xbf_d[ci * P:(ci + 1) * P, half:])
                nc.sync.dma_start(out=xb[:, 3, :half], in_=xbf_d[3 * P:4 * P, :half])
                nc.scalar.dma_start(out=xb[:, 3, half:], in_=xbf_d[3 * P:4 * P, half:])
                # weights on sync queue (k first), consts on gpsimd queue
                nc.sync.dma_start(out=wk8[:], in_=wk8_d[:].rearrange("(cc p) o -> p cc o", p=P))
                nc.sync.dma_start(out=wq8[:], in_=wq8_d[:].rearrange("(cc p) o -> p cc o", p=P))
                nc.sync.dma_start(out=wv8[:], in_=wv8_d[:].rearrange("(cc p) o -> p cc o", p=P))
                nc.sync.dma_start(out=wp8[:], in_=wp8_d[:].rearrange("(cc p) o -> p cc o", p=P))
                for t, d in (
                    (indh_sb, indh_d), (indt_sb, indt_d), (gamma_sb, gamma_d),
                    (beta_sb, beta_d), (bq8_sb, bq8_d), (bpe_sb, bpe_d),
                    (bcast16_sb, bcast16_d),
                ):
                    nc.gpsimd.dma_start(out=t[:], in_=d[:])

                scale_sb = gpool.tile([P, NCC], F32, tag="scale")
                shift_sb = gpool.tile([P, NCC], F32, tag="shift")
                gpsum = gppool.tile([P // GS, 2 * NCC], F32, tag="gstat")

                for ci in range(NCC):
                    # DVE: bn_stats over tokens 0..2047 (4 blocks of 512)
                    stats = gpool.tile([P, 4, 6], F32, tag="stats")
                    for sg in range(4):
                        nc.vector.bn_stats(
                            out=stats[:, sg, :],
                            in_=xb[:, ci, sg * 512:(sg + 1) * 512],
                        )
                    mv = gpool.tile([P, 2], F32, tag="mv")
                    nc.vector.bn_aggr(out=mv[:], in_=stats[:])
                    # u = [mean, E[x^2]] over the DVE half
                    u = gpool.tile([P, 2], F32, tag="u")
                    nc.vector.tensor_copy(out=u[:, 0:1], in_=mv[:, 0:1])
                    nc.vector.tensor_tensor(u[:, 1:2], mv[:, 0:1], mv[:, 0:1], ALU.mult)
                    nc.vector.tensor_add(u[:, 1:2], u[:, 1:2], mv[:, 1:2])
                    # ACT: raw sums over tokens 2048..3071
                    s_act = gpool.tile([P, 2], F32, tag="sact")
                    scr = gpool.tile([P, 1024], BF16, tag="scr")
                    nc.scalar.activation(
                        out=scr[:], in_=xb[:, ci, 2048:3072],
                        func=AF.Copy, accum_out=s_act[:, 0:1],
                    )
                    nc.scalar.activation(
                        out=scr[:], in_=xb[:, ci, 2048:3072],
                        func=AF.Square, accum_out=s_act[:, 1:2],
                    )
                    # Pool: raw sums over tokens 3072..4095
                    s_pool = gpool.tile([P, 2], F32, tag="spool")
                    scr2 = gpool.tile([P, 1024], BF16, tag="scr2")
                    nc.gpsimd.scalar_tensor_tensor(
                        out=scr2[:], in0=xb[:, ci, 3072:4096], scalar=1.0,
                        in1=xb[:, ci, 3072:4096], op0=ALU.mult, op1=ALU.bypass,
                        accum_out=s_pool[:, 0:1],
                    )
                    nc.gpsimd.scalar_tensor_tensor(
                        out=scr2[:], in0=xb[:, ci, 3072:4096], scalar=1.0,
                        in1=xb[:, ci, 3072:4096], op0=ALU.mult, op1=ALU.mult,
                        accum_out=s_pool[:, 1:2],
                    )
                    # group-reduce: indh has 1/32 (mean-halves), indt has 1/(16*4096)
                    gsl = gpsum[:, ci * 2:(ci + 1) * 2]
                    nc.tensor.matmul(gsl, lhsT=indh_sb[:], rhs=u[:], start=True, stop=False)
                    nc.tensor.matmul(gsl, lhsT=indt_sb[:], rhs=s_act[:], start=False, stop=False)
                    nc.tensor.matmul(gsl, lhsT=indt_sb[:], rhs=s_pool[:], start=False, stop=True)

                    # group mean/rstd -> broadcast -> per-channel scale/shift
                    gmr = gpool.tile([P // GS, 2], F32, tag="gmr", name=f"gmr{ci}")
                    nc.vector.tensor_copy(out=gmr[:], in_=gsl)
                    mu = gmr[:, 0:1]
                    var = gmr[:, 1:2]
                    tmpv = gpool.tile([P // GS, 1], F32, tag="tmpv")
                    nc.vector.tensor_tensor(tmpv[:], mu, mu, ALU.mult)
                    nc.vector.tensor_tensor(var, var, tmpv[:], ALU.subtract)
                    nc.scalar.activation(
                        out=var, in_=var, func=AF.Sqrt, bias=eps_sb[:], scale=1.0
                    )
                    nc.vector.reciprocal(out=var, in_=var)
                    bpsum = gppool.tile([P, 2], F32, tag="bc")
                    nc.tensor.matmul(
                        bpsum[:], lhsT=bcast16_sb[:], rhs=gmr[:], start=True, stop=True
                    )
                    sc = scale_sb[:, ci:ci + 1]
                    sh = shift_sb[:, ci:ci + 1]
                    nc.vector.tensor_tensor(
                        sc, bpsum[:, 1:2], gamma_sb[:, ci:ci + 1], ALU.mult
                    )
                    nc.vector.tensor_tensor(sh, bpsum[:, 0:1], sc, ALU.mult)
                    nc.vector.tensor_tensor(
                        sh, beta_sb[:, ci:ci + 1], sh, ALU.subtract
                    )
                    # h8 = x*sc + sh in fp8; halves on Pool + (ACT for ci<2 else DVE)
                    nc.gpsimd.tensor_scalar(
                        out=h8[:, ci, :half], in0=xb[:, ci, :half],
                        scalar1=sc, scalar2=sh, op0=ALU.mult, op1=ALU.add,
                    )
                    if ci < 2:
                        nc.scalar.activation(
                            out=h8[:, ci, half:], in_=xb[:, ci, half:],
                            func=AF.Identity, bias=sh, scale=sc,
                        )
                    else:
                        nc.vector.tensor_scalar(
                            out=h8[:, ci, half:], in0=xb[:, ci, half:],
                            scalar1=sc, scalar2=sh, op0=ALU.mult, op1=ALU.add,
                        )


            # ====== phase 1: QKV projections (DoubleRow fp8) ======
            # PSUM->SBUF fp8 conversion copies rotate over Pool/DVE/ACT (ACT is
            # idle during this phase); bias is folded into the copy where needed.
            ncpy = [0]

            def cpy3_engine():
                ncpy[0] += 1
                return (nc.gpsimd, nc.vector, nc.scalar)[ncpy[0] % 3]

            def copy_to(eng, dst, src, bias=None):
                if eng is nc.scalar:
                    if bias is None:
                        eng.activation(out=dst, in_=src, func=AF.Copy)
                    else:
                        eng.activation(out=dst, in_=src, func=AF.Identity, bias=bias)
                elif eng is nc.gpsimd:
                    eng.tensor_scalar(
                        out=dst, in0=src,
                        scalar1=(0.0 if bias is None else bias), scalar2=None,
                        op0=ALU.add,
                    )
                else:
                    if bias is None:
                        eng.tensor_copy(out=dst, in_=src)
                    else:
                        eng.tensor_scalar(
                            out=dst, in0=src, scalar1=bias, scalar2=None,
                            op0=ALU.add,
                        )

            with tc.tile_pool(name="mmp", bufs=4, space="PSUM") as mmpool:

                def emit_k(jtp):
                    # k for j tiles (2*jtp, 2*jtp+1), all out chunks
                    for oc in range(NCC):
                        ps = mmpool.tile([P, 2, 512], F32, tag="mm")
                        for t in range(2):
                            jt = jtp * 2 + t
                            for g in range(2):
                                nc.tensor.matmul(
                                    ps[:, t, :],
                                    lhsT=wk8[:, 2 * g:2 * g + 2, oc * P:(oc + 1) * P],
                                    rhs=h8[:, 2 * g:2 * g + 2, jt * 512:(jt + 1) * 512],
                                    start=(g == 0), stop=(g == 1), perf_mode=DR,
                                )
                        copy_to(
                            cpy3_engine(),
                            k8[:, oc, jtp * 1024:(jtp + 1) * 1024], ps[:, :, :],
                        )

                def emit_q(itp):
                    for oc in range(NCC):
                        ps = mmpool.tile([P, 2, 512], F32, tag="mm")
                        for t in range(2):
                            it = itp * 2 + t
                            for g in range(2):
                                nc.tensor.matmul(
                                    ps[:, t, :],
                                    lhsT=wq8[:, 2 * g:2 * g + 2, oc * P:(oc + 1) * P],
                                    rhs=h8[:, 2 * g:2 * g + 2, it * 512:(it + 1) * 512],
                                    start=(g == 0), stop=(g == 1), perf_mode=DR,
                                )
                        # copy + bq (x8) bias
                        copy_to(
                            cpy3_engine(),
                            q8[:, oc, itp * 1024:(itp + 1) * 1024], ps[:, :, :],
                            bias=bq8_sb[:, oc:oc + 1],
                        )

                def emit_v(jcp):
                    # vT for j chunks (2*jcp, 2*jcp+1)
                    ps = mmpool.tile([P, 2, 512], F32, tag="mm")
                    for t in range(2):
                        jc = jcp * 2 + t
                        for g in range(2):
                            nc.tensor.matmul(
                                ps[:, t, :],
                                lhsT=h8[:, 2 * g:2 * g + 2, jc * P:(jc + 1) * P],
                                rhs=wv8[:, 2 * g:2 * g + 2, :],
                                start=(g == 0), stop=(g == 1), perf_mode=DR,
                            )
                    copy_to(cpy3_engine(), vt8[:, jcp * 2:jcp * 2 + 2, :], ps[:, :, :])

                emit_k(0)
                emit_q(0)
                for jcp in range(0, 4):
                    emit_v(jcp)
                emit_k(1)
                for jcp in range(4, 8):
                    emit_v(jcp)
                emit_k(2)
                for jcp in range(8, 12):
                    emit_v(jcp)
                emit_k(3)
                for jcp in range(12, 16):
                    emit_v(jcp)
                emit_q(1)

            # ====== phase 2: attention + phase 3 projection ======
            with ExitStack() as stack1:
                stgpool = stack1.enter_context(tc.tile_pool(name="stg", bufs=3))
                etpool = stack1.enter_context(tc.tile_pool(name="et", bufs=3))
                obpool = stack1.enter_context(tc.tile_pool(name="ob", bufs=NBLK))
                lbpool = stack1.enter_context(tc.tile_pool(name="lb", bufs=2))
                lrbpool = stack1.enter_context(tc.tile_pool(name="lrb", bufs=2))
                stpool = stack1.enter_context(tc.tile_pool(name="stp", bufs=3, space="PSUM"))
                oapool = stack1.enter_context(tc.tile_pool(name="oap", bufs=1, space="PSUM"))
                lpool = stack1.enter_context(tc.tile_pool(name="lp", bufs=1, space="PSUM"))
                xrpool = stack1.enter_context(tc.tile_pool(name="xr", bufs=4))
                ospool = stack1.enter_context(tc.tile_pool(name="os", bufs=4))

                def attn_batch(ib, b, opsum, lpsum):
                    isl = slice(ib * 512, (ib + 1) * 512)
                    stage = stgpool.tile([P, 8, 512], BF16, tag="stage")
                    npool = 5 if b % 2 == 0 else 4
                    for g in range(8):
                        jc = b * 8 + g
                        st = stpool.tile([P, 512], F32, tag="st")
                        for gg in range(2):
                            nc.tensor.matmul(
                                st[:],
                                lhsT=k8[:, 2 * gg:2 * gg + 2, jc * P:(jc + 1) * P],
                                rhs=q8[:, 2 * gg:2 * gg + 2, isl],
                                start=(gg == 0), stop=(gg == 1), perf_mode=DR,
                            )
                        copy_to(nc.gpsimd if g < npool else nc.vector,
                                stage[:, g, :], st[:])
                    et = etpool.tile([P, 8, 512], F8, tag="et")
                    nc.scalar.activation(
                        out=et[:], in_=stage[:], func=AF.Exp,
                        scale=EXP_SCALE, bias=ebias_sb[:],
                    )
                    for p in range(4):
                        pair = b * 4 + p
                        jc0 = pair * 2
                        first = pair == 0
                        last = pair == 15
                        for cc in range(NCC):
                            nc.tensor.matmul(
                                opsum[cc][:],
                                lhsT=vt8[:, jc0:jc0 + 2, cc * P:(cc + 1) * P],
                                rhs=et[:, 2 * p:2 * p + 2, :],
                                start=first, stop=last, perf_mode=DR,
                            )
                        nc.tensor.matmul(
                            lpsum[:],
                            lhsT=ones8_sb[:],
                            rhs=et[:, 2 * p:2 * p + 2, :],
                            start=first, stop=last, perf_mode=DR,
                        )

                def attn_tail(ib, opsum, lpsum):
                    # drain O psum at a fixed 1/64 scale (no data dependency on l,
                    # so the next i-block's O matmuls are not held up); the 1/l
                    # normalization happens in phase 3 via lrb
                    obf = obpool.tile([P, NCC, 512], F8, tag="obf", name=f"obf{ib}")
                    for cc in range(NCC):
                        eng = nc.vector if cc % 2 == 0 else nc.gpsimd
                        if eng is nc.vector:
                            eng.tensor_scalar(
                                out=obf[:, cc, :], in0=opsum[cc][:],
                                scalar1=1.0 / 64.0, scalar2=None, op0=ALU.mult,
                            )
                        else:
                            eng.tensor_scalar(
                                out=obf[:, cc, :], in0=opsum[cc][:],
                                scalar1=1.0 / 64.0, scalar2=None, op0=ALU.mult,
                            )
                    linv = lbpool.tile([1, 512], F32, tag="linv")
                    nc.vector.reciprocal(out=linv[:], in_=lpsum[:])
                    lrb = lrbpool.tile([P, 512], F32, tag="lrb", name=f"lrb{ib}")
                    nc.gpsimd.partition_broadcast(lrb[:, :], linv[0:1, :], channels=P)
                    return obf, lrb

                def emit_proj(ib, obf, lrb):
                    # phase 3 for one i-block: out = (Wp @ O)*lrb + (x + bp_eff)
                    isl = slice(ib * 512, (ib + 1) * 512)
                    for oc in range(NCC):
                        xr = xrpool.tile([P, 512], F32, tag="xr")
                        nc.sync.dma_start(
                            out=xr[:], in_=xres_d[oc * P:(oc + 1) * P, isl]
                        )
                        nc.gpsimd.tensor_scalar(
                            out=xr[:], in0=xr[:], scalar1=bpe_sb[:, oc:oc + 1],
                            scalar2=None, op0=ALU.add,
                        )
                        ps = stpool.tile([P, 512], F32, tag="st")
                        for g in range(2):
                            nc.tensor.matmul(
                                ps[:],
                                lhsT=wp8[:, 2 * g:2 * g + 2, oc * P:(oc + 1) * P],
                                rhs=obf[:, 2 * g:2 * g + 2, :],
                                start=(g == 0), stop=(g == 1), perf_mode=DR,
                            )
                        tmp = ospool.tile([P, 512], F32, tag="tmp")
                        nc.gpsimd.tensor_tensor(tmp[:], ps[:], lrb[:, :], ALU.mult)
                        ost = ospool.tile([P, 512], F32, tag="ost")
                        nc.vector.tensor_tensor(ost[:], tmp[:], xr[:], ALU.add)
                        eng = nc.sync if oc % 2 == 0 else nc.gpsimd
                        eng.dma_start(out=y_d[oc * P:(oc + 1) * P, isl], in_=ost[:])

                o_bfs = []
                for ib in range(NBLK):
                    opsum = [
                        oapool.tile([P, 512], F32, tag=f"o{cc}", name=f"op{cc}")
                        for cc in range(NCC)
                    ]
                    lpsum = lpool.tile([1, 512], F32, tag="l")
                    for b in range(4):
                        attn_batch(ib, b, opsum, lpsum)
                        # interleave previous block's projection behind batch 1
                        if b == 1 and ib > 0:
                            emit_proj(ib - 1, *o_bfs[ib - 1])
                    o_bfs.append(attn_tail(ib, opsum, lpsum))
                emit_proj(NBLK - 1, *o_bfs[NBLK - 1])

    if split_waits:
        _split_excess_waits(nc)
    return nc


_NC = None


def _get_nc():
    global _NC
    if _NC is None:
        _NC = build_nc()
    return _NC


def _core0_feed(inputs):
    """Input map for core 0 (batch 0, first query half) — used by test harnesses."""
    maps = _build_in_maps(**inputs)
    return maps[0]


def _build_in_maps(x, gamma, beta, Wq, bq, Wk, bk, Wv, bv, Wp, bp):
    x = np.asarray(x, dtype=np.float32)
    B, c, H, W = x.shape
    assert (B, c, H, W) == (4, C, 64, 64)

    def pc(v):  # [C] -> [P, NCC]
        return np.ascontiguousarray(np.asarray(v, np.float32).reshape(NCC, P).T)

    indh = np.zeros((P, P // GS), np.float32)
    indh[np.arange(P), np.arange(P) // GS] = 1.0 / (GS * 2.0)
    indt = np.zeros((P, P // GS), np.float32)
    indt[np.arange(P), np.arange(P) // GS] = 1.0 / (GS * HW)
    bcast16 = np.zeros((P // GS, P), np.float32)
    bcast16[np.arange(P) // GS, np.arange(P)] = 1.0

    Wp32 = np.asarray(Wp, np.float32)
    bv32 = np.asarray(bv, np.float32)
    bp_eff = np.asarray(bp, np.float32) + Wp32 @ bv32

    def w8(wmat):
        return np.ascontiguousarray(
            np.asarray(wmat, np.float32).T * WS
        ).astype(E4)

    shared = {
        "wq8": w8(Wq), "wk8": w8(Wk), "wv8": w8(Wv), "wp8": w8(Wp),
        "bq8_pc": pc(np.asarray(bq, np.float32) * WS),
        "bpe_pc": pc(bp_eff),
        "gamma_pc": pc(gamma), "beta_pc": pc(beta),
        "indh": indh, "indt": indt, "bcast16": bcast16,
    }

    xf = x.reshape(B, C, HW)
    in_maps = []
    for core in range(8):
        b, halfsel = divmod(core, 2)
        xb = xf[b]
        if halfsel == 0:
            x_bc = xb
        else:
            x_bc = np.concatenate([xb[:, IHALF:], xb[:, :IHALF]], axis=1)
        x_bc = np.ascontiguousarray(x_bc)
        in_maps.append({
            "x_bf": x_bc.astype(BF),
            "x_res": np.ascontiguousarray(x_bc[:, :IHALF]),
            **shared,
        })
    return in_maps


def kernel(x, gamma, beta, Wq, bq, Wk, bk, Wv, bv, Wp, bp):
    nc = _get_nc()
    in_maps = _build_in_maps(x, gamma, beta, Wq, bq, Wk, bk, Wv, bv, Wp, bp)

    from concourse.bass_utils import run_bass_kernel_spmd

    res = run_bass_kernel_spmd(nc, in_maps, list(range(8)))

    B = 4
    out = np.empty((B, C, HW), np.float32)
    for core in range(8):
        b, halfsel = divmod(core, 2)
        out[b, :, halfsel * IHALF:(halfsel + 1) * IHALF] = res.results[core]["yout"]
    return out.reshape(B, C, 64, 64)


# revision 18
# speedup vs baseline: 2.6748x; 1.0892x over previous
"""AttnBlock (GroupNorm -> single-head attention over 64x64 tokens -> proj -> residual)
for Trainium2, SPMD over 8 NeuronCores.

Sharding: core = batch(4) x query-half(2) (token order along j is permutation-
invariant for softmax attention and GroupNorm stats).

All heavy matmuls run in fp8e4m3 with DoubleRow perf mode (contract 256/instr
at 0.5 cycles/row): QKV projections, S^T = k^T q, O = vT e, the softmax
denominator (ones-matmul), and the output projection.

Scaling scheme (all powers of 2, exact):
  weights stored as 8*W^T in fp8; q,k,v carry x8; S_psum = 64*S_true
  exp: et = exp(S_psum * 1/(64*sqrt(C)) - ln16) = e_true/16  (fp8 range safe)
  l_psum = sum(et)/8 = l_true/128 ; lrb = recip = 128/l_true
  o_bf = opsum * lrb = 64*O_norm (fp8) ; proj psum = 512*(Wp O_norm)
  out = ps*(1/512) + (x + bp + Wp bv)

Bias folds: bk dropped exactly (softmax shift invariance); bv folded into
bp_eff = bp + Wp@bv host-side; bq added on the q PSUM->SBUF copy.

Softmax exp is staged: S psum tiles are copied (Pool/DVE) to a bf16 SBUF
buffer of 8 j-chunks, then ONE 4096-wide ACT exp produces fp8 et directly.

Layouts (SBUF, partition dim first):
  h8,k8: [128, 4cc, 4096] channel on partitions, tokens free (fp8)
  q8   : [128, 4cc, 2048]
  vt8  : [128jc, 32, 512] token chunk on partitions, channel free (fp8)
  S^T  : psum [128 j, 512 i]; et: [128 j, 8jc, 512 i] fp8
  O    : psum [128 c, 512 i] accumulated over 16 j-pairs via DoubleRow
"""

import math
import numpy as np
import ml_dtypes

import concourse.bass as bass
import concourse.mybir as mybir
import concourse.tile as tile
from concourse import library_config

P = 128
C = 512
NCC = C // P          # 4 channel chunks
HW = 4096             # tokens per batch image
IHALF = 2048          # query tokens per core
NBLK = IHALF // 512   # 4 i-blocks of 512
NJC = HW // P         # 32 j chunks of 128
NJT = HW // 512       # 8 j tiles of 512
GS = 16               # channels per group
EPS = 1e-6
WS = 8.0
EXP_SCALE = 1.0 / (64.0 * math.sqrt(C))
EXP_BIAS = -math.log(16.0)

F32 = mybir.dt.float32
BF16 = mybir.dt.bfloat16
F8 = mybir.dt.float8e4
BF = ml_dtypes.bfloat16
E4 = ml_dtypes.float8_e4m3

DR = mybir.MatmulPerfMode.DoubleRow
ALU = mybir.AluOpType
AF = mybir.ActivationFunctionType


def _split_excess_waits(nc):
    """walrus in this container accepts only ONE sync-wait per instruction;
    move extra waits onto same-engine NOPs placed immediately before."""
    for fn in nc.m.functions:
        for bb in fn.blocks:
            insts = list(bb.instructions)
            out = []
            changed = False
            for inst in insts:
                si = inst.sync_info
                if si is not None and len(si.on_wait) > 1:
                    waits = list(si.on_wait)
                    for k, w in enumerate(waits[:-1]):
                        nop = mybir.InstNoOp(
                            name=f"{inst.name}-ws{k}",
                            sync_info=mybir.SyncInfo(on_wait=[w], on_update=[]),
                            bass_nofuse=True,
                            engine=inst.engine,
                        )
                        out.append(nop)
                    inst.sync_info = mybir.SyncInfo(
                        on_wait=[waits[-1]], on_update=list(si.on_update)
                    )
                    changed = True
                out.append(inst)
            if changed:
                bb.instructions = out


def build_nc(split_waits=True):
    nc = bass.Bass()

    xbf_d = nc.declare_dram_parameter("x_bf", [C, HW], BF16, isOutput=False)
    xres_d = nc.declare_dram_parameter("x_res", [C, IHALF], F32, isOutput=False)
    wq8_d = nc.declare_dram_parameter("wq8", [C, C], F8, isOutput=False)
    wk8_d = nc.declare_dram_parameter("wk8", [C, C], F8, isOutput=False)
    wv8_d = nc.declare_dram_parameter("wv8", [C, C], F8, isOutput=False)
    wp8_d = nc.declare_dram_parameter("wp8", [C, C], F8, isOutput=False)
    bq8_d = nc.declare_dram_parameter("bq8_pc", [P, NCC], F32, isOutput=False)
    bpe_d = nc.declare_dram_parameter("bpe_pc", [P, NCC], F32, isOutput=False)
    gamma_d = nc.declare_dram_parameter("gamma_pc", [P, NCC], F32, isOutput=False)
    beta_d = nc.declare_dram_parameter("beta_pc", [P, NCC], F32, isOutput=False)
    indh_d = nc.declare_dram_parameter("indh", [P, P // GS], F32, isOutput=False)
    indt_d = nc.declare_dram_parameter("indt", [P, P // GS], F32, isOutput=False)
    bcast16_d = nc.declare_dram_parameter("bcast16", [P // GS, P], F32, isOutput=False)
    y_d = nc.declare_dram_parameter("yout", [C, IHALF], F32, isOutput=True)

    from contextlib import ExitStack

    with tile.TileContext(nc) as tc:
        with ExitStack() as stack:
            wpool = stack.enter_context(tc.tile_pool(name="w", bufs=1))
            cpool = stack.enter_context(tc.tile_pool(name="const", bufs=1))
            hpool = stack.enter_context(tc.tile_pool(name="hbuf", bufs=1))
            kpool = stack.enter_context(tc.tile_pool(name="kbuf", bufs=1))
            vpool = stack.enter_context(tc.tile_pool(name="vbuf", bufs=1))
            qpool = stack.enter_context(tc.tile_pool(name="qbuf", bufs=1))
            wq8 = wpool.tile([P, NCC, C], F8, tag="wq8")
            wk8 = wpool.tile([P, NCC, C], F8, tag="wk8")
            wv8 = wpool.tile([P, NCC, C], F8, tag="wv8")
            wp8 = wpool.tile([P, NCC, C], F8, tag="wp8")

            bq8_sb = cpool.tile([P, NCC], F32, tag="bq8")
            bpe_sb = cpool.tile([P, NCC], F32, tag="bpe")
            gamma_sb = cpool.tile([P, NCC], F32, tag="gamma")
            beta_sb = cpool.tile([P, NCC], F32, tag="beta")
            indh_sb = cpool.tile([P, P // GS], F32, tag="indh")
            indt_sb = cpool.tile([P, P // GS], F32, tag="indt")
            bcast16_sb = cpool.tile([P // GS, P], F32, tag="bcast16")
            eps_sb = cpool.tile([P // GS, 1], F32, tag="eps")
            ebias_sb = cpool.tile([P, 1], F32, tag="ebias")
            ones8_sb = cpool.tile([P, 2, 1], F8, tag="ones8")

            h8 = hpool.tile([P, NCC, HW], F8, tag="h8")
            k8 = kpool.tile([P, NCC, HW], F8, tag="k8")
            vt8 = vpool.tile([P, NJC, C], F8, tag="vt8")
            q8 = qpool.tile([P, NCC, IHALF], F8, tag="q8")

            # gpsimd custom-op library (partition_broadcast)
            nc.gpsimd.load_library(library_config.proxy)
            nc.vector.memset(eps_sb[:], EPS)
            nc.vector.memset(ebias_sb[:], EXP_BIAS)
            nc.vector.memset(ones8_sb[:], 1.0)

            # ====== phase 0: DMA in, GN stats on 3 engines, h8 = fp8(x*sc+sh) ======
            with ExitStack() as stack0:
                xpool = stack0.enter_context(tc.tile_pool(name="xbuf", bufs=1))
                gpool = stack0.enter_context(tc.tile_pool(name="gn", bufs=2))
                gppool = stack0.enter_context(tc.tile_pool(name="gnp", bufs=2, space="PSUM"))
                xb = xpool.tile([P, NCC, HW], BF16, tag="xb")
                half = HW // 2
                # one chunk per DMA queue; two halves each so stats can start early
                for ci, eng in ((0, nc.sync), (1, nc.gpsimd), (2, nc.scalar)):
                    eng.dma_start(out=xb[:, ci, :half], in_=xbf_d[ci * P:(ci + 1) * P, :half])
                    eng.dma_start(out=xb[:, ci, half:], in_=xbf_d[ci * P:(ci + 1) * P, half:])
                nc.sync.dma_start(out=xb[:, 3, :half], in_=xbf_d[3 * P:4 * P, :half])
                nc.scalar.dma_start(out=xb[:, 3, half:], in_=xbf_d[3 * P:4 * P, half:])
                # weights on sync queue (k first), consts on gpsimd queue
                nc.sync.dma_start(out=wk8[:], in_=wk8_d[:].rearrange("(cc p) o -> p cc o", p=P))
                nc.sync.dma_start(out=wq8[:], in_=wq8_d[:].rearrange("(cc p) o -> p cc o", p=P))
                nc.sync.dma_start(out=wv8[:], in_=wv8_d[:].rearrange("(cc p) o -> p cc o", p=P))
                nc.sync.dma_start(out=wp8[:], in_=wp8_d[:].rearrange("(cc p) o -> p cc o", p=P))
                for t, d in (
                    (indh_sb, indh_d), (indt_sb, indt_d), (gamma_sb, gamma_d),
                    (beta_sb, beta_d), (bq8_sb, bq8_d), (bpe_sb, bpe_d),
                    (bcast16_sb, bcast16_d),
                ):
                    nc.gpsimd.dma_start(out=t[:], in_=d[:])

                scale_sb = gpool.tile([P, NCC], F32, tag="scale")
                shift_sb = gpool.tile([P, NCC], F32, tag="shift")
                gpsum = gppool.tile([P // GS, 2 * NCC], F32, tag="gstat")

                for ci in range(NCC):
                    # DVE: bn_stats over tokens 0..2047 (4 blocks of 512)
                    stats = gpool.tile([P, 4, 6], F32, tag="stats")
                    for sg in range(4):
                        nc.vector.bn_stats(
                            out=stats[:, sg, :],
                            in_=xb[:, ci, sg * 512:(sg + 1) * 512],
                        )
                    mv = gpool.tile([P, 2], F32, tag="mv")
                    nc.vector.bn_aggr(out=mv[:], in_=stats[:])
                    # u = [mean, E[x^2]] over the DVE half
                    u = gpool.tile([P, 2], F32, tag="u")
                    nc.vector.tensor_copy(out=u[:, 0:1], in_=mv[:, 0:1])
                    nc.vector.tensor_tensor(u[:, 1:2], mv[:, 0:1], mv[:, 0:1], ALU.mult)
                    nc.vector.tensor_add(u[:, 1:2], u[:, 1:2], mv[:, 1:2])
                    # ACT: raw sums over tokens 2048..3071
                    s_act = gpool.tile([P, 2], F32, tag="sact")
                    scr = gpool.tile([P, 1024], BF16, tag="scr")
                    nc.scalar.activation(
                        out=scr[:], in_=xb[:, ci, 2048:3072],
                        func=AF.Copy, accum_out=s_act[:, 0:1],
                    )
                    nc.scalar.activation(
                        out=scr[:], in_=xb[:, ci, 2048:3072],
                        func=AF.Square, accum_out=s_act[:, 1:2],
                    )
                    # Pool: raw sums over tokens 3072..4095
                    s_pool = gpool.tile([P, 2], F32, tag="spool")
                    scr2 = gpool.tile([P, 1024], BF16, tag="scr2")
                    nc.gpsimd.scalar_tensor_tensor(
                        out=scr2[:], in0=xb[:, ci, 3072:4096], scalar=1.0,
                        in1=xb[:, ci, 3072:4096], op0=ALU.mult, op1=ALU.bypass,
                        accum_out=s_pool[:, 0:1],
                    )
                    nc.gpsimd.scalar_tensor_tensor(
                        out=scr2[:], in0=xb[:, ci, 3072:4096], scalar=1.0,
                        in1=xb[:, ci, 3072:4096], op0=ALU.mult, op1=ALU.mult,
                        accum_out=s_pool[:, 1:2],
                    )
                    # group-reduce: indh has 1/32 (mean-halves), indt has 1/(16*4096)
                    gsl = gpsum[:, ci * 2:(ci + 1) * 2]
                    nc.tensor.matmul(gsl, lhsT=indh_sb[:], rhs=u[:], start=True, stop=False)
                    nc.tensor.matmul(gsl, lhsT=indt_sb[:], rhs=s_act[:], start=False, stop=False)
                    nc.tensor.matmul(gsl, lhsT=indt_sb[:], rhs=s_pool[:], start=False, stop=True)

                    # group mean/rstd -> broadcast -> per-channel scale/shift
                    gmr = gpool.tile([P // GS, 2], F32, tag="gmr", name=f"gmr{ci}")
                    nc.vector.tensor_copy(out=gmr[:], in_=gsl)
                    mu = gmr[:, 0:1]
                    var = gmr[:, 1:2]
                    tmpv = gpool.tile([P // GS, 1], F32, tag="tmpv")
                    nc.vector.tensor_tensor(tmpv[:], mu, mu, ALU.mult)
                    nc.vector.tensor_tensor(var, var, tmpv[:], ALU.subtract)
                    nc.scalar.activation(
                        out=var, in_=var, func=AF.Sqrt, bias=eps_sb[:], scale=1.0
                    )
                    nc.vector.reciprocal(out=var, in_=var)
                    bpsum = gppool.tile([P, 2], F32, tag="bc")
                    nc.tensor.matmul(
                        bpsum[:], lhsT=bcast16_sb[:], rhs=gmr[:], start=True, stop=True
                    )
                    sc = scale_sb[:, ci:ci + 1]
                    sh = shift_sb[:, ci:ci + 1]
                    nc.vector.tensor_tensor(
                        sc, bpsum[:, 1:2], gamma_sb[:, ci:ci + 1], ALU.mult
                    )
                    nc.vector.tensor_tensor(sh, bpsum[:, 0:1], sc, ALU.mult)
                    nc.vector.tensor_tensor(
                        sh, beta_sb[:, ci:ci + 1], sh, ALU.subtract
                    )
                    # h8 = x*sc + sh in fp8; halves on Pool + (ACT for ci<2 else DVE)
                    nc.gpsimd.tensor_scalar(
                        out=h8[:, ci, :half], in0=xb[:, ci, :half],
                        scalar1=sc, scalar2=sh, op0=ALU.mult, op1=ALU.add,
                    )
                    if ci < 2:
                        nc.scalar.activation(
                            out=h8[:, ci, half:], in_=xb[:, ci, half:],
                            func=AF.Identity, bias=sh, scale=sc,
                        )
                    else:
                        nc.vector.tensor_scalar(
                            out=h8[:, ci, half:], in0=xb[:, ci, half:],
                            scalar1=sc, scalar2=sh, op0=ALU.mult, op1=ALU.add,
                        )


            # ====== phase 1: QKV projections (DoubleRow fp8) ======
            # PSUM->SBUF fp8 conversion copies rotate over Pool/DVE/ACT (ACT is
            # idle during this phase); bias is folded into the copy where needed.
            ncpy = [0]

            def cpy3_engine():
                ncpy[0] += 1
                return (nc.gpsimd, nc.vector, nc.scalar)[ncpy[0] % 3]

            def copy_to(eng, dst, src, bias=None):
                if eng is nc.scalar:
                    if bias is None:
                        eng.activation(out=dst, in_=src, func=AF.Copy)
                    else:
                        eng.activation(out=dst, in_=src, func=AF.Identity, bias=bias)
                elif eng is nc.gpsimd:
                    eng.tensor_scalar(
                        out=dst, in0=src,
                        scalar1=(0.0 if bias is None else bias), scalar2=None,
                        op0=ALU.add,
                    )
                else:
                    if bias is None:
                        eng.tensor_copy(out=dst, in_=src)
                    else:
                        eng.tensor_scalar(
                            out=dst, in0=src, scalar1=bias, scalar2=None,
                            op0=ALU.add,
                        )

            with tc.tile_pool(name="mmp", bufs=4, space="PSUM") as mmpool:

                def emit_k(jtp):
                    for oc in range(NCC):
                        ps = mmpool.tile([P, 2, 512], F32, tag="mm")
                        for t in range(2):
                            jt = jtp * 2 + t
                            for g in range(2):
                                nc.tensor.matmul(
                                    ps[:, t, :],
                                    lhsT=wk8[:, 2 * g:2 * g + 2, oc * P:(oc + 1) * P],
                                    rhs=h8[:, 2 * g:2 * g + 2, jt * 512:(jt + 1) * 512],
                                    start=(g == 0), stop=(g == 1), perf_mode=DR,
                                )
                        copy_to(
                            cpy3_engine(),
                            k8[:, oc, jtp * 1024:(jtp + 1) * 1024], ps[:, :, :],
                        )

                def emit_q(itp):
                    for oc in range(NCC):
                        ps = mmpool.tile([P, 2, 512], F32, tag="mm")
                        for t in range(2):
                            it = itp * 2 + t
                            for g in range(2):
                                nc.tensor.matmul(
                                    ps[:, t, :],
                                    lhsT=wq8[:, 2 * g:2 * g + 2, oc * P:(oc + 1) * P],
                                    rhs=h8[:, 2 * g:2 * g + 2, it * 512:(it + 1) * 512],
                                    start=(g == 0), stop=(g == 1), perf_mode=DR,
                                )
                        copy_to(
                            cpy3_engine(),
                            q8[:, oc, itp * 1024:(itp + 1) * 1024], ps[:, :, :],
                            bias=bq8_sb[:, oc:oc + 1],
                        )

                def emit_v(jcp):
                    ps = mmpool.tile([P, 2, 512], F32, tag="mm")
                    for t in range(2):
                        jc = jcp * 2 + t
                        for g in range(2):
                            nc.tensor.matmul(
                                ps[:, t, :],
                                lhsT=h8[:, 2 * g:2 * g + 2, jc * P:(jc + 1) * P],
                                rhs=wv8[:, 2 * g:2 * g + 2, :],
                                start=(g == 0), stop=(g == 1), perf_mode=DR,
                            )
                    copy_to(cpy3_engine(), vt8[:, jcp * 2:jcp * 2 + 2, :], ps[:, :, :])

                emit_k(0)
                emit_q(0)
                for jcp in range(0, 4):
                    emit_v(jcp)
                emit_k(1)
                for jcp in range(4, 8):
                    emit_v(jcp)
                emit_k(2)
                for jcp in range(8, 12):
                    emit_v(jcp)
                emit_k(3)
                for jcp in range(12, 16):
                    emit_v(jcp)
                emit_q(1)

            # ====== phase 2: attention (+ phase 3 interleaved per i-block) ======
            with ExitStack() as stack1:
                stgpool = stack1.enter_context(tc.tile_pool(name="stg", bufs=3))
                etpool = stack1.enter_context(tc.tile_pool(name="et", bufs=4))
                obpool = stack1.enter_context(tc.tile_pool(name="ob", bufs=NBLK))
                lbpool = stack1.enter_context(tc.tile_pool(name="lb", bufs=2))
                lrbpool = stack1.enter_context(tc.tile_pool(name="lrb", bufs=2))
                stpool = stack1.enter_context(tc.tile_pool(name="stp", bufs=3, space="PSUM"))
                oapool = stack1.enter_context(tc.tile_pool(name="oap", bufs=1, space="PSUM"))
                lpool = stack1.enter_context(tc.tile_pool(name="lp", bufs=1, space="PSUM"))
                xrpool = stack1.enter_context(tc.tile_pool(name="xr", bufs=4))
                ospool = stack1.enter_context(tc.tile_pool(name="os", bufs=4))

                def attn_batch(ib, b, opsum, lpsum):
                    # two half-batches of 4 j-chunks: a 2048-wide exp per half
                    # shortens the S -> copies -> exp -> O dependency chain
                    isl = slice(ib * 512, (ib + 1) * 512)
                    for hb in range(2):
                        stage = stgpool.tile([P, 4, 512], BF16, tag="stage")
                        for g in range(4):
                            jc = b * 8 + hb * 4 + g
                            st = stpool.tile([P, 512], F32, tag="st")
                            for gg in range(2):
                                nc.tensor.matmul(
                                    st[:],
                                    lhsT=k8[:, 2 * gg:2 * gg + 2, jc * P:(jc + 1) * P],
                                    rhs=q8[:, 2 * gg:2 * gg + 2, isl],
                                    start=(gg == 0), stop=(gg == 1), perf_mode=DR,
                                )
                            copy_to(nc.gpsimd if g < 2 + ((b + hb) % 2) else nc.vector,
                                    stage[:, g, :], st[:])
                        et = etpool.tile([P, 4, 512], F8, tag="et")
                        nc.scalar.activation(
                            out=et[:], in_=stage[:], func=AF.Exp,
                            scale=EXP_SCALE, bias=ebias_sb[:],
                        )
                        for p in range(2):
                            pair = b * 4 + hb * 2 + p
                            jc0 = pair * 2
                            first = pair == 0
                            last = pair == 15
                            for cc in range(NCC):
                                nc.tensor.matmul(
                                    opsum[cc][:],
                                    lhsT=vt8[:, jc0:jc0 + 2, cc * P:(cc + 1) * P],
                                    rhs=et[:, 2 * p:2 * p + 2, :],
                                    start=first, stop=last, perf_mode=DR,
                                )
                            nc.tensor.matmul(
                                lpsum[:],
                                lhsT=ones8_sb[:],
                                rhs=et[:, 2 * p:2 * p + 2, :],
                                start=first, stop=last, perf_mode=DR,
                            )

                def attn_tail(ib, opsum, lpsum):
                    # drain O psum at a fixed 1/64 scale (no dependency on l);
                    # 1/l is applied in phase 3 via lrb
                    obf = obpool.tile([P, NCC, 512], F8, tag="obf", name=f"obf{ib}")
                    for cc in range(NCC):
                        eng = nc.vector if cc % 2 == 0 else nc.gpsimd
                        eng.tensor_scalar(
                            out=obf[:, cc, :], in0=opsum[cc][:],
                            scalar1=1.0 / 64.0, scalar2=None, op0=ALU.mult,
                        )
                    linv = lbpool.tile([1, 512], F32, tag="linv")
                    nc.vector.reciprocal(out=linv[:], in_=lpsum[:])
                    lrb = lrbpool.tile([P, 512], F32, tag="lrb", name=f"lrb{ib}")
                    nc.gpsimd.partition_broadcast(lrb[:, :], linv[0:1, :], channels=P)
                    return obf, lrb

                def emit_proj(ib, obf, lrb):
                    # phase 3 for one i-block: out = (Wp @ O)*lrb + (x + bp_eff)
                    isl = slice(ib * 512, (ib + 1) * 512)
                    for oc in range(NCC):
                        xr = xrpool.tile([P, 512], F32, tag="xr")
                        nc.sync.dma_start(
                            out=xr[:], in_=xres_d[oc * P:(oc + 1) * P, isl]
                        )
                        nc.gpsimd.tensor_scalar(
                            out=xr[:], in0=xr[:], scalar1=bpe_sb[:, oc:oc + 1],
                            scalar2=None, op0=ALU.add,
                        )
                        ps = stpool.tile([P, 512], F32, tag="st")
                        for g in range(2):
                            nc.tensor.matmul(
                                ps[:],
                                lhsT=wp8[:, 2 * g:2 * g + 2, oc * P:(oc + 1) * P],
                                rhs=obf[:, 2 * g:2 * g + 2, :],
                                start=(g == 0), stop=(g == 1), perf_mode=DR,
                            )
                        tmp = ospool.tile([P, 512], F32, tag="tmp")
                        nc.gpsimd.tensor_tensor(tmp[:], ps[:], lrb[:, :], ALU.mult)
                        ost = ospool.tile([P, 512], F32, tag="ost")
                        nc.vector.tensor_tensor(ost[:], tmp[:], xr[:], ALU.add)
                        eng = nc.sync if oc % 2 == 0 else nc.gpsimd
                        eng.dma_start(out=y_d[oc * P:(oc + 1) * P, isl], in_=ost[:])

                o_bfs = []
                for ib in range(NBLK):
                    opsum = [
                        oapool.tile([P, 512], F32, tag=f"o{cc}", name=f"op{cc}")
                        for cc in range(NCC)
                    ]
                    lpsum = lpool.tile([1, 512], F32, tag="l")
                    for b in range(4):
                        attn_batch(ib, b, opsum, lpsum)
                        if b == 1 and ib > 0:
                            emit_proj(ib - 1, *o_bfs[ib - 1])
                    o_bfs.append(attn_tail(ib, opsum, lpsum))
                emit_proj(NBLK - 1, *o_bfs[NBLK - 1])

    if split_waits:
        _split_excess_waits(nc)
    return nc


_NC = None


def _get_nc():
    global _NC
    if _NC is None:
        _NC = build_nc()
    return _NC


def _core0_feed(inputs):
    """Input map for core 0 (batch 0, first query half) — used by test harnesses."""
    maps = _build_in_maps(**inputs)
    return maps[0]


def _build_in_maps(x, gamma, beta, Wq, bq, Wk, bk, Wv, bv, Wp, bp):
    x = np.asarray(x, dtype=np.float32)
    B, c, H, W = x.shape
    assert (B, c, H, W) == (4, C, 64, 64)

    def pc(v):  # [C] -> [P, NCC]
        return np.ascontiguousarray(np.asarray(v, np.float32).reshape(NCC, P).T)

    indh = np.zeros((P, P // GS), np.float32)
    indh[np.arange(P), np.arange(P) // GS] = 1.0 / (GS * 2.0)
    indt = np.zeros((P, P // GS), np.float32)
    indt[np.arange(P), np.arange(P) // GS] = 1.0 / (GS * HW)
    bcast16 = np.zeros((P // GS, P), np.float32)
    bcast16[np.arange(P) // GS, np.arange(P)] = 1.0

    Wp32 = np.asarray(Wp, np.float32)
    bv32 = np.asarray(bv, np.float32)
    bp_eff = np.asarray(bp, np.float32) + Wp32 @ bv32

    def w8(wmat):
        return np.ascontiguousarray(
            np.asarray(wmat, np.float32).T * WS
        ).astype(E4)

    shared = {
        "wq8": w8(Wq), "wk8": w8(Wk), "wv8": w8(Wv), "wp8": w8(Wp),
        "bq8_pc": pc(np.asarray(bq, np.float32) * WS),
        "bpe_pc": pc(bp_eff),
        "gamma_pc": pc(gamma), "beta_pc": pc(beta),
        "indh": indh, "indt": indt, "bcast16": bcast16,
    }

    xf = x.reshape(B, C, HW)
    in_maps = []
    for core in range(8):
        b, halfsel = divmod(core, 2)
        xb = xf[b]
        if halfsel == 0:
            x_bc = xb
        else:
            x_bc = np.concatenate([xb[:, IHALF:], xb[:, :IHALF]], axis=1)
        x_bc = np.ascontiguousarray(x_bc)
        in_maps.append({
            "x_bf": x_bc.astype(BF),
            "x_res": np.ascontiguousarray(x_bc[:, :IHALF]),
            **shared,
        })
    return in_maps


def kernel(x, gamma, beta, Wq, bq, Wk, bk, Wv, bv, Wp, bp):
    nc = _get_nc()
    in_maps = _build_in_maps(x, gamma, beta, Wq, bq, Wk, bk, Wv, bv, Wp, bp)

    from concourse.bass_utils import run_bass_kernel_spmd

    res = run_bass_kernel_spmd(nc, in_maps, list(range(8)))

    B = 4
    out = np.empty((B, C, HW), np.float32)
    for core in range(8):
        b, halfsel = divmod(core, 2)
        out[b, :, halfsel * IHALF:(halfsel + 1) * IHALF] = res.results[core]["yout"]
    return out.reshape(B, C, 64, 64)
